# revision 20
# baseline (speedup 1.0000x reference)
"""Trainium2 Bass kernel for nn_MultiHeadAttention (B=2, S=2048, D=1024, H=16).

Sharding: 8 cores = 2 batches x 4 head-groups (4 heads per core, tensor
parallel over heads). Each core computes, for its batch b and its 4 heads:
  QT/KT = (x @ W.T).T projections in transposed layout [256, 2048]
  V     = value @ wv.T in normal layout, augmented with a ones column (Z trick)
  E^T   = exp(scoresT) tiles [k,q] directly from matmul (no max subtraction;
          scores are O(1) for this module so exp is safe, and masked entries
          use a multiplicative 0/1 mask so they are exactly 0)
  outT  = V_aug.T @ E^T accumulated over k tiles -> row 64 carries Z = sum(E)
  ffT   = wff_rows-partial @ (attn_outT * 1/Z) + bff/4   as [1024, 2048]
Host sums the 4 partial ffT per batch and transposes back.

Performance structure:
  - A dummy-matmul warmup chain at t=0 keeps the PE HAM clock-gate at
    2.4 GHz from ~4us (otherwise the first ~50us run at 1.2 GHz).
  - Score matmuls for a head PAIR are packed into PE row-tiles (0,0) and
    (64,0) (contraction is DH=64), doubling score throughput.
  - The softmax Z is broadcast across partitions with a rank-1 fp16 PE
    matmul (no DRAM round-trip); 1/Z runs at full 128-lane DVE width and
    the normalize-multiply is fused with the psum->sbuf cast.
  - AV matmuls trim fully-masked leading columns (causal).
  - x loads stream per 512-token group (half-K granularity) so the first
    projection starts ~2us in; output is written in bf16.
  - Projections of group g+1 and ff of group g-1 are queued as filler
    units between attention matmuls to cover the exp latency; ff units
    are deferred (lazy queue) to the late, filler-poor attention groups.
"""

import sys

sys.path.insert(0, "/opt/trn_rl_repo")

import ml_dtypes
import numpy as np

import concourse.bass as bass
import concourse.mybir as mybir
import concourse.tile as tile
from concourse import bacc
from concourse.bass_utils import run_bass_kernel_spmd

P = 128
B, S, D, H = 2, 2048, 1024, 16
DH = D // H  # 64
NCORES = 8
GPB = NCORES // B  # cores (head groups) per batch = 4
HPC = H // GPB  # heads per core = 4
HD = HPC * DH  # projected cols per core = 256
F32 = mybir.dt.float32
F32R = mybir.dt.float32r
FP16 = mybir.dt.float16
BF16 = mybir.dt.bfloat16
QGW = 512  # q-group width (psum free dim)
AF = mybir.ActivationFunctionType
NPBF16 = ml_dtypes.bfloat16
NWARM = 16  # warmup matmuls to engage the HAM clock un-throttle

_PROG_CACHE: dict = {}


def build_program(variant: str, use_bias: bool, s=S, d=D, hpc=HPC,
                  n_devices=NCORES):
    """variant: 'causal' | 'dense' | 'generic'. Returns compiled Bacc."""
    assert variant in ("causal", "dense", "generic")
    kc_n = d // P           # contraction chunks over model dim
    tt = s // P             # token tiles
    hd = hpc * DH           # per-core projected width
    dc_n = hd // P          # dout chunks for QT/KT (and hd chunks for ff)
    tg_n = s // QGW         # token/q groups
    tpg = QGW // P          # token tiles per group (4)
    khalf = kc_n // 2

    nc = bacc.Bacc("TRN2", target_bir_lowering=False, debug=False,
                   num_devices=n_devices)

    def din(name, shape, dt=BF16):
        return nc.dram_tensor(name, list(shape), dt, kind="ExternalInput").ap()

    xqT = din("xqT", (d, s))
    xkT = din("xkT", (d, s))
    xvT = din("xvT", (d, s))
    wqT = din("wqT", (d, hd))   # pre-scaled by 1/sqrt(DH) on host
    wkT = din("wkT", (d, hd))
    wvT = din("wvT", (d, hd))
    wffT = din("wffT", (hd, d))
    if use_bias:
        bq = din("bq", (hd,), F32)   # pre-scaled by 1/sqrt(DH) on host
        bk = din("bk", (hd,), F32)
        bv = din("bv", (1, hd))
        bffq = din("bffq", (d,), F32)    # bff / GPB
        onesb = din("onesb", (1, P))
    if variant == "causal":
        dmask = din("dmask", (P, P))  # [k, q]: 1 if k <= q else 0
    if variant == "generic":
        mbT = din("mbT", (s, s), F32)  # mask[b,0].T * -1e9, [k, q] layout
    outT = nc.dram_tensor("outT", [d, s], BF16, kind="ExternalOutput").ap()

    with tile.TileContext(nc) as tc:
        with (
            nc.allow_low_precision(reason="bf16 matmul chain; psum stays fp32"),
            tc.tile_pool(name="consts", bufs=1) as consts,
            tc.tile_pool(name="xin", bufs=1) as xin,
            tc.tile_pool(name="acts", bufs=1) as acts,
            tc.tile_pool(name="epool", bufs=8) as epool,
            tc.tile_pool(name="opool", bufs=4) as opool,
            tc.tile_pool(name="zp", bufs=4) as zp,
            tc.tile_pool(name="ps", bufs=1, space="PSUM") as ps,
        ):
            # ---- warmup: engage the PE clock un-throttle immediately.
            # K=1 matmuls on a single-partition row (fast 1-lane memset on
            # DVE) still stream N columns, so they warm the PE at full rate.
            warm = consts.tile([1, P + QGW], BF16, tag="warm")
            nc.vector.memset(warm[:], 0.125)
            wmp = ps.tile([P, QGW], F32, tag="opacc", bufs=2, name="wmp")
            for i in range(NWARM):
                nc.tensor.matmul(wmp[:], lhsT=warm[0:1, :P],
                                 rhs=warm[0:1, P:],
                                 start=(i == 0), stop=(i == NWARM - 1))
            warm_out = consts.tile([1, QGW], BF16, tag="warmout")
            nc.vector.tensor_copy(warm_out[0:1, :], wmp[0:1, :])
            # preload the scalar engine's exp table during the warmup so the
            # first real exp doesn't pay the ~2.7us ACT_TABLE_LOAD
            expdummy = consts.tile([1, 8], BF16, tag="expdummy")
            nc.scalar.activation(expdummy[0:1, :], warm[0:1, :8], AF.Exp)

            # ones row for the rank-1 1/Z partition-broadcast
            ones_sb = consts.tile([1, P], FP16, tag="ones")
            nc.gpsimd.memset(ones_sb[:], 1.0)

            # ---- constant / weight tiles ----
            wq_sb = consts.tile([P, kc_n, hd], BF16, tag="wq")
            wk_sb = consts.tile([P, kc_n, hd], BF16, tag="wk")
            wv_sb = consts.tile([P, kc_n, hd], BF16, tag="wv")
            wff_sb = consts.tile([P, dc_n, d], BF16, tag="wff")
            _loaded = set()

            def load_w(name, sb, dram, half=None):
                key = (name, half)
                if key in _loaded:
                    return
                _loaded.add(key)
                src = dram.rearrange("(c p) m -> p c m", p=P)
                if half is None:
                    nc.sync.dma_start(sb[:], src)
                else:
                    k0 = half * khalf
                    nc.sync.dma_start(sb[:, k0:k0 + khalf, :],
                                      src[:, k0:k0 + khalf, :])
            if use_bias:
                bq_sb = consts.tile([P, dc_n], F32, tag="bq")
                bk_sb = consts.tile([P, dc_n], F32, tag="bk")
                nc.sync.dma_start(bq_sb[:], bq.rearrange("(c p) -> p c", p=P))
                nc.sync.dma_start(bk_sb[:], bk.rearrange("(c p) -> p c", p=P))
                bv_sb = consts.tile([1, hd], BF16, tag="bv")
                nc.sync.dma_start(bv_sb[:], bv[:])
                bffq_sb = consts.tile([P, kc_n], F32, tag="bffq")
                nc.sync.dma_start(bffq_sb[:],
                                  bffq.rearrange("(c p) -> p c", p=P))
                onesb_sb = consts.tile([1, P], BF16, tag="onesb")
                nc.sync.dma_start(onesb_sb[:], onesb[:])
            if variant == "causal":
                dmask_sb = consts.tile([P, P], BF16, tag="dmask")
                nc.sync.dma_start(dmask_sb[:], dmask[:])

            # per-group activation tiles
            xq_g = [acts.tile([P, kc_n, QGW], BF16, tag=f"xq{g}",
                              name=f"xq_{g}") for g in range(tg_n)]
            xk_g = [acts.tile([P, kc_n, QGW], BF16, tag=f"xk{g}",
                              name=f"xk_{g}") for g in range(tg_n)]
            qT_g = [acts.tile([P, dc_n, QGW], BF16, tag=f"qT{g}",
                              name=f"qT_{g}") for g in range(tg_n)]
            kT_g = [acts.tile([P, dc_n, QGW], BF16, tag=f"kT{g}",
                              name=f"kT_{g}") for g in range(tg_n)]
            va_g = [acts.tile([P, tpg, hpc * (DH + 1)], BF16, tag=f"va{g}",
                              name=f"va_{g}") for g in range(tg_n)]
            at_g = [acts.tile([P, dc_n, QGW], BF16, tag=f"at{g}",
                              name=f"at_{g}") for g in range(tg_n)]

            _xdma_done = set()

            def load_x(name, g, x_sb, x_dram):
                """Load activations for one token group. Group 0 is split
                into half-K DMAs so the first projection chain starts
                sooner; later groups use one DMA to cut SP issue cost."""
                if (name, g) in _xdma_done:
                    return
                _xdma_done.add((name, g))
                src = x_dram.rearrange("(c p) m -> p c m",
                                       p=P)[:, :, g * QGW:(g + 1) * QGW]
                if g == 0:
                    for half in range(2):
                        k0 = half * khalf
                        nc.sync.dma_start(x_sb[:, k0:k0 + khalf, :],
                                          src[:, k0:k0 + khalf, :])
                else:
                    nc.sync.dma_start(x_sb[:], src)

            def proj_qk_units(tg, w_sb, x_sb, b_sb, dest, out, dcs=None):
                """Append filler units: 2 half-chains per dc."""
                cell = {}
                for dc in (range(dc_n) if dcs is None else dcs):
                    for half in range(2):
                        def chain(tg=tg, dc=dc, half=half, w_sb=w_sb,
                                  x_sb=x_sb, b_sb=b_sb, dest=dest):
                            if half == 0:
                                cell[dc] = ps.tile([P, QGW], F32, tag="pacc",
                                                   bufs=2,
                                                   name=f"pp_{tg}_{dc}")
                            pp = cell[dc]
                            k0 = half * khalf
                            for kc in range(k0, k0 + khalf):
                                nc.tensor.matmul(
                                    pp[:],
                                    lhsT=w_sb[:, kc, dc * P:(dc + 1) * P],
                                    rhs=x_sb[:, kc, :],
                                    start=(kc == 0),
                                    stop=(kc == kc_n - 1),
                                )
                            if half == 1:
                                if use_bias:
                                    nc.scalar.activation(
                                        dest[:, dc, :], pp[:], AF.Identity,
                                        bias=b_sb[:, dc:dc + 1])
                                else:
                                    nc.vector.tensor_copy(dest[:, dc, :],
                                                          pp[:])
                        out.append(chain)

            def proj_v_units(tg, out):
                def ones_unit(tg=tg):
                    nc.gpsimd.memset(
                        va_g[tg].rearrange("p t (h e) -> p t h e",
                                           e=DH + 1)[:, :, :, DH], 1.0)
                    xvt = xin.tile([P, kc_n, QGW], BF16, tag="xvstream",
                                   bufs=2, name=f"xvt_{tg}")
                    nc.sync.dma_start(
                        xvt[:],
                        xvT.rearrange("(c p) m -> p c m",
                                      p=P)[:, :, tg * QGW:(tg + 1) * QGW])
                    ones_unit.xvt = xvt
                out.append(ones_unit)
                for ti in range(tpg):
                    def v_unit(tg=tg, ti=ti, holder=ones_unit):
                        xvt = holder.xvt
                        vp = ps.tile([P, QGW], F32, tag="pacc", bufs=2,
                                     name=f"vp_{tg}_{ti}")
                        if use_bias:
                            nc.tensor.matmul(vp[:, :hd],
                                             lhsT=onesb_sb[0:1, :],
                                             rhs=bv_sb[:, :], start=True,
                                             stop=False)
                        for kc in range(kc_n):
                            nc.tensor.matmul(
                                vp[:, :hd],
                                lhsT=xvt[:, kc, ti * P:(ti + 1) * P],
                                rhs=wv_sb[:, kc, :],
                                start=(kc == 0 and not use_bias),
                                stop=(kc == kc_n - 1),
                            )
                        nc.vector.tensor_copy(
                            va_g[tg][:, ti].rearrange(
                                "p (h e) -> p h e", e=DH + 1)[:, :, :DH],
                            vp[:, :hd].rearrange("p (h e) -> p h e", e=DH))
                    out.append(v_unit)

            def ff_units(qg, out, tail=False):
                for nck in range(kc_n):
                    def ff_unit(qg=qg, nck=nck, tail=tail):
                        fp = ps.tile([P, QGW], F32, tag="pacc", bufs=2,
                                     name=f"fp_{nck}_{qg}")
                        for dc in range(dc_n):
                            nc.tensor.matmul(
                                fp[:],
                                lhsT=wff_sb[:, dc, nck * P:(nck + 1) * P],
                                rhs=at_g[qg][:, dc, :],
                                start=(dc == 0),
                                stop=(dc == dc_n - 1),
                            )
                        ot = opool.tile([P, QGW], BF16, tag="otile",
                                        name=f"ot_{nck}_{qg}")
                        if use_bias:
                            nc.scalar.activation(ot[:], fp[:], AF.Identity,
                                                 bias=bffq_sb[:, nck:nck + 1])
                        elif tail:
                            # exp is done by now; use the idle scalar engine
                            nc.scalar.copy(ot[:], fp[:])
                        else:
                            nc.vector.tensor_copy(ot[:], fp[:])
                        nc.sync.dma_start(
                            outT[nck * P:(nck + 1) * P,
                                 qg * QGW:(qg + 1) * QGW], ot[:])
                    out.append(ff_unit)

            # two filler tiers: pf must drain before the next attention
            # group (projections); lf (ff) can slip to late groups.
            from collections import deque
            pf = deque()
            lf = deque()

            def run_units(n=None):
                k = (len(pf) + len(lf)) if n is None else n
                for _ in range(k):
                    if pf:
                        pf.popleft()()
                    elif lf:
                        lf.popleft()()
                    else:
                        return

            def drain_pf():
                while pf:
                    pf.popleft()()

            def attention(qg):
                kmax = (qg + 1) * tpg if variant == "causal" else tt
                for hp in range(hpc // 2):
                    dch = hp
                    h0 = 2 * hp
                    op_h = [
                        ps.tile([P, QGW], F32, tag="opacc", bufs=2,
                                name=f"op{j}_{hp}_{qg}")
                        for j in range(2)
                    ]
                    ets = {}

                    def emit_scores(kt, hp=hp, dch=dch, ets=ets, qg=qg):
                        off = (max(0, kt * P - qg * QGW)
                               if variant == "causal" else 0)
                        kg, kx = divmod(kt, tpg)
                        sp = ps.tile([P, 2, QGW], F32, tag="mmw", bufs=2,
                                     name=f"sp_{hp}_{qg}_{kt}")
                        for j in range(2):
                            r = j * DH
                            nc.tensor.matmul(
                                sp[:, j, off:],
                                lhsT=kT_g[kg][r:r + DH, dch,
                                              kx * P:(kx + 1) * P],
                                rhs=qT_g[qg][r:r + DH, dch, off:],
                                start=True,
                                stop=True,
                            )
                        if variant == "generic":
                            mb_sb = xin.tile([P, QGW], F32, tag="mstream",
                                             bufs=4,
                                             name=f"mb_{hp}_{qg}_{kt}")
                            nc.sync.dma_start(
                                mb_sb[:],
                                mbT[kt * P:(kt + 1) * P,
                                    qg * QGW:(qg + 1) * QGW])
                            for j in range(2):
                                nc.vector.tensor_add(
                                    sp[:, j, :], sp[:, j, :], mb_sb[:])
                        et = epool.tile([P, 2, QGW], BF16, tag="etile",
                                        name=f"et_{hp}_{qg}_{kt}")
                        # one ACTIVATE covers both heads even when the
                        # leading `off` columns are trimmed (3D AP)
                        nc.scalar.activation(et[:, :, off:], sp[:, :, off:],
                                             AF.Exp)
                        if variant == "causal" and kt * P - qg * QGW >= 0:
                            doff = kt * P - qg * QGW
                            for j in range(2):
                                nc.vector.tensor_mul(
                                    et[:, j, doff:doff + P],
                                    et[:, j, doff:doff + P],
                                    dmask_sb[:])
                        ets[kt] = (et, off)

                    def emit_av(kt, hp=hp, ets=ets, qg=qg, kmax=kmax,
                                op_h=op_h, h0=h0):
                        et, off = ets.pop(kt)
                        kg, kx = divmod(kt, tpg)
                        for j in range(2):
                            h = h0 + j
                            nc.tensor.matmul(
                                op_h[j][:DH + 1, off:],
                                lhsT=va_g[kg][:, kx, h * (DH + 1):
                                              (h + 1) * (DH + 1)],
                                rhs=et[:, j, off:],
                                start=(kt == 0),
                                stop=(kt == kmax - 1),
                            )

                    emit_scores(0)
                    for kt in range(1, kmax):
                        emit_scores(kt)
                        run_units(1)
                        emit_av(kt - 1)
                    emit_av(kmax - 1)
                    run_units(1)

                    # normalize: broadcast Z across partitions with a rank-1
                    # fp16 matmul, take 1/Z at full 128-lane width, and fuse
                    # the psum->sbuf cast with the multiply.
                    for j in range(2):
                        h = h0 + j
                        op = op_h[j]
                        po = (h * DH) % P
                        zrh = zp.tile([1, QGW], FP16, tag="zrh",
                                      name=f"zrh_{h}_{qg}")
                        nc.vector.tensor_copy(zrh[0:1, :], op[DH:DH + 1, :])
                        zbz = ps.tile([P, QGW], F32, tag="pacc", bufs=2,
                                      name=f"zbz_{h}_{qg}")
                        nc.tensor.matmul(
                            zbz[:],
                            lhsT=ones_sb[0:1, :],
                            rhs=zrh[0:1, :],
                            start=True, stop=True)
                        # custom-DVE ops ignore input base partitions, but
                        # this one reads a full base-0 tile (verified OK)
                        zbi = zp.tile([P, QGW], F32, tag="zbi", bufs=2,
                                      name=f"zbi_{h}_{qg}")
                        nc.vector.reciprocal_approx_fast(zbi[:], zbz[:])
                        nc.vector.tensor_mul(
                            at_g[qg][po:po + DH, dch, :],
                            op[:DH, :],
                            zbi[po:po + DH, :])

            # ---- schedule over token groups ----
            def queue_proj(tg, first=False):
                bqs = bq_sb if use_bias else None
                bks = bk_sb if use_bias else None
                if first:
                    # head-pair-0 (dc 0) projections first so attention(0)
                    # can start before pair-1's chains run (as fillers)
                    pf.append(lambda: load_w("wq", wq_sb, wqT, 0))
                    pf.append(lambda tg=tg: load_x("xq", tg, xq_g[tg], xqT))
                    pf.append(lambda: load_w("wq", wq_sb, wqT, 1))
                    proj_qk_units(tg, wq_sb, xq_g[tg], bqs, qT_g[tg], pf,
                                  dcs=(0,))
                    pf.append(lambda: load_w("wk", wk_sb, wkT))
                    pf.append(lambda tg=tg: load_x("xk", tg, xk_g[tg], xkT))
                    proj_qk_units(tg, wk_sb, xk_g[tg], bks, kT_g[tg], pf,
                                  dcs=(0,))
                    pf.append(lambda: load_w("wv", wv_sb, wvT))
                    proj_v_units(tg, pf)
                    proj_qk_units(tg, wq_sb, xq_g[tg], bqs, qT_g[tg], pf,
                                  dcs=(1,))
                    proj_qk_units(tg, wk_sb, xk_g[tg], bks, kT_g[tg], pf,
                                  dcs=(1,))
                    return
                pf.append(lambda tg=tg: load_x("xq", tg, xq_g[tg], xqT))
                proj_qk_units(tg, wq_sb, xq_g[tg], bqs, qT_g[tg], pf)
                pf.append(lambda tg=tg: load_x("xk", tg, xk_g[tg], xkT))
                proj_qk_units(tg, wk_sb, xk_g[tg], bks, kT_g[tg], pf)
                proj_v_units(tg, pf)

            if variant == "causal":
                queue_proj(0, first=True)
                # drain all but the 4 dc-1 chains; those become fillers
                # inside attention(0) and only gate head-pair 1
                run_units(len(pf) - 4)
                for tg in range(tg_n):
                    if tg + 1 < tg_n:
                        queue_proj(tg + 1)
                    lf.append(lambda: load_w("wff", wff_sb, wffT))
                    if tg > 0:
                        ff_units(tg - 1, lf)
                    attention(tg)
                    drain_pf()
                ff_units(tg_n - 1, lf, tail=True)
                run_units()
            else:
                queue_proj(0, first=True)
                for tg in range(1, tg_n):
                    queue_proj(tg)
                drain_pf()
                lf.append(lambda: load_w("wff", wff_sb, wffT))
                for qg in range(tg_n):
                    if qg > 0:
                        ff_units(qg - 1, lf)
                    attention(qg)
                ff_units(tg_n - 1, lf, tail=True)
                run_units()

    nc.compile()
    return nc


def _classify_mask(mask: np.ndarray) -> str:
    m = np.asarray(mask)[:, 0]  # [B, S, S]
    if not m.any():
        return "dense"
    s = m.shape[-1]
    causal = np.triu(np.ones((s, s), dtype=m.dtype), k=1)
    if all(np.array_equal(m[b], causal) for b in range(m.shape[0])):
        return "causal"
    return "generic"


def _bf(x):
    return np.ascontiguousarray(np.ascontiguousarray(x).astype(NPBF16))


def _make_in_maps(variant, query, key, value, mask, wq, bq, wk, bk, wv, bv,
                  wff, bff, use_bias):
    scale = np.float32(1.0 / np.sqrt(np.float32(DH)))
    wqTs = _bf((wq * scale).T)
    wkT = _bf(wk.T)
    wvT = _bf(wv.T)
    wffT = _bf(wff.T)

    qT = [_bf(query[b].T) for b in range(B)]
    kT = [_bf(key[b].T) for b in range(B)]
    vT = [_bf(value[b].T) for b in range(B)]
    mbT = None
    if variant == "generic":
        mbT = [np.ascontiguousarray(mask[b, 0].T * np.float32(-1e9))
               for b in range(B)]

    dmask = np.tril(np.ones((P, P), np.float32)).T

    in_maps = []
    for c in range(NCORES):
        b, hg = c // GPB, c % GPB
        sl = slice(hg * HD, (hg + 1) * HD)
        m = {
            "xqT": qT[b], "xkT": kT[b], "xvT": vT[b],
            "wqT": np.ascontiguousarray(wqTs[:, sl]),
            "wkT": np.ascontiguousarray(wkT[:, sl]),
            "wvT": np.ascontiguousarray(wvT[:, sl]),
            "wffT": np.ascontiguousarray(wffT[sl, :]),
        }
        if use_bias:
            m["bq"] = np.ascontiguousarray((bq * scale)[sl]).astype(np.float32)
            m["bk"] = np.ascontiguousarray(bk[sl]).astype(np.float32)
            m["bv"] = _bf(bv[sl])[None, :]
            m["bffq"] = (bff / GPB).astype(np.float32)
            m["onesb"] = np.ones((1, P), NPBF16)
        if variant == "causal":
            m["dmask"] = _bf(dmask)
        if variant == "generic":
            m["mbT"] = mbT[b]
        in_maps.append(m)
    return in_maps


def kernel(**inputs) -> np.ndarray:
    query = np.ascontiguousarray(inputs["query"], dtype=np.float32)
    key = np.ascontiguousarray(inputs["key"], dtype=np.float32)
    value = np.ascontiguousarray(inputs["value"], dtype=np.float32)
    mask = np.asarray(inputs["mask"], dtype=np.float32)
    wq = np.asarray(inputs["wq"], np.float32)
    bq = np.asarray(inputs["bq"], np.float32)
    wk = np.asarray(inputs["wk"], np.float32)
    bk = np.asarray(inputs["bk"], np.float32)
    wv = np.asarray(inputs["wv"], np.float32)
    bv = np.asarray(inputs["bv"], np.float32)
    wff = np.asarray(inputs["wff"], np.float32)
    bff = np.asarray(inputs["bff"], np.float32)

    variant = _classify_mask(mask)
    use_bias = bool(bq.any() or bk.any() or bv.any() or bff.any())
    pkey = (variant, use_bias)
    if pkey not in _PROG_CACHE:
        _PROG_CACHE[pkey] = build_program(variant, use_bias)
    nc = _PROG_CACHE[pkey]

    in_maps = _make_in_maps(variant, query, key, value, mask, wq, bq, wk, bk,
                            wv, bv, wff, bff, use_bias)
    res = run_bass_kernel_spmd(nc, in_maps, core_ids=list(range(NCORES)))
    out = np.empty((B, S, D), np.float32)
    for b in range(B):
        acc = res.results[b * GPB]["outT"].astype(np.float32)
        for g in range(1, GPB):
            acc = acc + res.results[b * GPB + g]["outT"].astype(np.float32)
        out[b] = acc.T
    return out


if __name__ == "__main__":
    import reference

    inputs = {k: np.asarray(v) for k, v in reference.setup_inputs().items()}
    out = kernel(**inputs)
    print("kernel out:", out.shape, out.dtype)


# revision 21
# speedup vs baseline: 1.0069x; 1.0069x over previous
"""Trainium2 Bass kernel for nn_MultiHeadAttention (B=2, S=2048, D=1024, H=16).

Sharding: 8 cores = 2 batches x 4 head-groups (4 heads per core, tensor
parallel over heads). Each core computes, for its batch b and its 4 heads:
  QT/KT = (x @ W.T).T projections in transposed layout [256, 2048]
  V     = value @ wv.T in normal layout, augmented with a ones column (Z trick)
  E^T   = exp(scoresT) tiles [k,q] directly from matmul (no max subtraction;
          scores are O(1) for this module so exp is safe, and masked entries
          use a multiplicative 0/1 mask so they are exactly 0)
  outT  = V_aug.T @ E^T accumulated over k tiles -> row 64 carries Z = sum(E)
  ffT   = wff_rows-partial @ (attn_outT * 1/Z) + bff/4   as [1024, 2048]
Host sums the 4 partial ffT per batch and transposes back.

Performance structure:
  - A dummy-matmul warmup chain at t=0 keeps the PE HAM clock-gate at
    2.4 GHz from ~4us (otherwise the first ~50us run at 1.2 GHz).
  - Score matmuls for a head PAIR are packed into PE row-tiles (0,0) and
    (64,0) (contraction is DH=64), doubling score throughput.
  - The softmax Z is broadcast across partitions with a rank-1 fp16 PE
    matmul (no DRAM round-trip); 1/Z runs at full 128-lane DVE width and
    the normalize-multiply is fused with the psum->sbuf cast.
  - AV matmuls trim fully-masked leading columns (causal).
  - x loads stream per 512-token group (half-K granularity) so the first
    projection starts ~2us in; output is written in bf16.
  - Projections of group g+1 and ff of group g-1 are queued as filler
    units between attention matmuls to cover the exp latency; ff units
    are deferred (lazy queue) to the late, filler-poor attention groups.
"""

import sys

sys.path.insert(0, "/opt/trn_rl_repo")

import ml_dtypes
import numpy as np

import concourse.bass as bass
import concourse.mybir as mybir
import concourse.tile as tile
from concourse import bacc
from concourse.bass_utils import run_bass_kernel_spmd

P = 128
B, S, D, H = 2, 2048, 1024, 16
DH = D // H  # 64
NCORES = 8
GPB = NCORES // B  # cores (head groups) per batch = 4
HPC = H // GPB  # heads per core = 4
HD = HPC * DH  # projected cols per core = 256
F32 = mybir.dt.float32
F32R = mybir.dt.float32r
FP16 = mybir.dt.float16
BF16 = mybir.dt.bfloat16
QGW = 512  # q-group width (psum free dim)
AF = mybir.ActivationFunctionType
NPBF16 = ml_dtypes.bfloat16
NWARM = 12  # warmup matmuls to engage the HAM clock un-throttle

_PROG_CACHE: dict = {}


def build_program(variant: str, use_bias: bool, s=S, d=D, hpc=HPC,
                  n_devices=NCORES):
    """variant: 'causal' | 'dense' | 'generic'. Returns compiled Bacc."""
    assert variant in ("causal", "dense", "generic")
    kc_n = d // P           # contraction chunks over model dim
    tt = s // P             # token tiles
    hd = hpc * DH           # per-core projected width
    dc_n = hd // P          # dout chunks for QT/KT (and hd chunks for ff)
    tg_n = s // QGW         # token/q groups
    tpg = QGW // P          # token tiles per group (4)
    khalf = kc_n // 2

    nc = bacc.Bacc("TRN2", target_bir_lowering=False, debug=False,
                   num_devices=n_devices)

    def din(name, shape, dt=BF16):
        return nc.dram_tensor(name, list(shape), dt, kind="ExternalInput").ap()

    # activations/weights arrive pre-tiled partition-major from the host
    # ([...,P, c, m] contiguous) so every DMA is few large descriptor lines
    xqT = din("xqT", (tg_n, P, kc_n, QGW))
    xkT = din("xkT", (tg_n, P, kc_n, QGW))
    xvT = din("xvT", (tg_n, P, kc_n, QGW))
    wqT = din("wqT", (P, kc_n, hd))   # pre-scaled by 1/sqrt(DH) on host
    wkT = din("wkT", (P, kc_n, hd))
    wvT = din("wvT", (P, kc_n, hd))
    wffT = din("wffT", (P, dc_n, d))
    if use_bias:
        bq = din("bq", (hd,), F32)   # pre-scaled by 1/sqrt(DH) on host
        bk = din("bk", (hd,), F32)
        bv = din("bv", (1, hd))
        bffq = din("bffq", (d,), F32)    # bff / GPB
        onesb = din("onesb", (1, P))
    if variant == "causal":
        dmask = din("dmask", (P, P))  # [k, q]: 1 if k <= q else 0
    if variant == "generic":
        mbT = din("mbT", (s, s), F32)  # mask[b,0].T * -1e9, [k, q] layout
    outT = nc.dram_tensor("outT", [d, s], BF16, kind="ExternalOutput").ap()

    with tile.TileContext(nc) as tc:
        with (
            nc.allow_low_precision(reason="bf16 matmul chain; psum stays fp32"),
            tc.tile_pool(name="consts", bufs=1) as consts,
            tc.tile_pool(name="xin", bufs=1) as xin,
            tc.tile_pool(name="acts", bufs=1) as acts,
            tc.tile_pool(name="epool", bufs=8) as epool,
            tc.tile_pool(name="opool", bufs=4) as opool,
            tc.tile_pool(name="zp", bufs=4) as zp,
            tc.tile_pool(name="ps", bufs=1, space="PSUM") as ps,
        ):
            # ---- warmup: engage the PE clock un-throttle immediately.
            # Must be full-K matmuls: K=1 ones do not register as PE
            # activity for the HAM monitor (measured: stayed cold to 37us).
            warm = consts.tile([P, P + QGW], BF16, tag="warm")
            nc.vector.memset(warm[:], 0.125)
            wmp = ps.tile([P, QGW], F32, tag="opacc", bufs=2, name="wmp")
            for i in range(NWARM):
                nc.tensor.matmul(wmp[:], lhsT=warm[:, :P],
                                 rhs=warm[:, P:],
                                 start=(i == 0), stop=(i == NWARM - 1))
            warm_out = consts.tile([1, QGW], BF16, tag="warmout")
            nc.vector.tensor_copy(warm_out[0:1, :], wmp[0:1, :])
            # preload the scalar engine's exp table during the warmup so the
            # first real exp doesn't pay the ~2.7us ACT_TABLE_LOAD
            expdummy = consts.tile([1, 8], BF16, tag="expdummy")
            nc.scalar.activation(expdummy[0:1, :], warm[0:1, :8], AF.Exp)

            # ones row for the rank-1 1/Z partition-broadcast
            ones_sb = consts.tile([1, P], FP16, tag="ones")
            nc.gpsimd.memset(ones_sb[:], 1.0)

            # ---- constant / weight tiles ----
            wq_sb = consts.tile([P, kc_n, hd], BF16, tag="wq")
            wk_sb = consts.tile([P, kc_n, hd], BF16, tag="wk")
            wv_sb = consts.tile([P, kc_n, hd], BF16, tag="wv")
            wff_sb = consts.tile([P, dc_n, d], BF16, tag="wff")
            _loaded = set()

            def load_w(name, sb, dram, half=None):
                key = (name, half)
                if key in _loaded:
                    return
                _loaded.add(key)
                if half is None:
                    nc.sync.dma_start(sb[:], dram)
                else:
                    k0 = half * khalf
                    nc.sync.dma_start(sb[:, k0:k0 + khalf, :],
                                      dram[:, k0:k0 + khalf, :])
            if use_bias:
                bq_sb = consts.tile([P, dc_n], F32, tag="bq")
                bk_sb = consts.tile([P, dc_n], F32, tag="bk")
                nc.sync.dma_start(bq_sb[:], bq.rearrange("(c p) -> p c", p=P))
                nc.sync.dma_start(bk_sb[:], bk.rearrange("(c p) -> p c", p=P))
                bv_sb = consts.tile([1, hd], BF16, tag="bv")
                nc.sync.dma_start(bv_sb[:], bv[:])
                bffq_sb = consts.tile([P, kc_n], F32, tag="bffq")
                nc.sync.dma_start(bffq_sb[:],
                                  bffq.rearrange("(c p) -> p c", p=P))
                onesb_sb = consts.tile([1, P], BF16, tag="onesb")
                nc.sync.dma_start(onesb_sb[:], onesb[:])
            if variant == "causal":
                dmask_sb = consts.tile([P, P], BF16, tag="dmask")
                nc.sync.dma_start(dmask_sb[:], dmask[:])

            # per-group activation tiles
            xq_g = [acts.tile([P, kc_n, QGW], BF16, tag=f"xq{g}",
                              name=f"xq_{g}") for g in range(tg_n)]
            xk_g = [acts.tile([P, kc_n, QGW], BF16, tag=f"xk{g}",
                              name=f"xk_{g}") for g in range(tg_n)]
            qT_g = [acts.tile([P, dc_n, QGW], BF16, tag=f"qT{g}",
                              name=f"qT_{g}") for g in range(tg_n)]
            kT_g = [acts.tile([P, dc_n, QGW], BF16, tag=f"kT{g}",
                              name=f"kT_{g}") for g in range(tg_n)]
            va_g = [acts.tile([P, tpg, hpc * (DH + 1)], BF16, tag=f"va{g}",
                              name=f"va_{g}") for g in range(tg_n)]
            at_g = [acts.tile([P, dc_n, QGW], BF16, tag=f"at{g}",
                              name=f"at_{g}") for g in range(tg_n)]

            _xdma_done = set()

            def load_x(name, g, x_sb, x_dram):
                """Load activations for one token group. Group 0 is split
                into half-K DMAs so the first projection chain starts
                sooner; later groups use one DMA to cut SP issue cost."""
                if (name, g) in _xdma_done:
                    return
                _xdma_done.add((name, g))
                src = x_dram[g]
                if g == 0:
                    for half in range(2):
                        k0 = half * khalf
                        nc.sync.dma_start(x_sb[:, k0:k0 + khalf, :],
                                          src[:, k0:k0 + khalf, :])
                else:
                    nc.sync.dma_start(x_sb[:], src)

            def proj_qk_units(tg, w_sb, x_sb, b_sb, dest, out, dcs=None):
                """Append filler units: 2 half-chains per dc."""
                cell = {}
                for dc in (range(dc_n) if dcs is None else dcs):
                    for half in range(2):
                        def chain(tg=tg, dc=dc, half=half, w_sb=w_sb,
                                  x_sb=x_sb, b_sb=b_sb, dest=dest):
                            if half == 0:
                                cell[dc] = ps.tile([P, QGW], F32, tag="pacc",
                                                   bufs=2,
                                                   name=f"pp_{tg}_{dc}")
                            pp = cell[dc]
                            k0 = half * khalf
                            for kc in range(k0, k0 + khalf):
                                nc.tensor.matmul(
                                    pp[:],
                                    lhsT=w_sb[:, kc, dc * P:(dc + 1) * P],
                                    rhs=x_sb[:, kc, :],
                                    start=(kc == 0),
                                    stop=(kc == kc_n - 1),
                                )
                            if half == 1:
                                if use_bias:
                                    nc.scalar.activation(
                                        dest[:, dc, :], pp[:], AF.Identity,
                                        bias=b_sb[:, dc:dc + 1])
                                else:
                                    nc.vector.tensor_copy(dest[:, dc, :],
                                                          pp[:])
                        out.append(chain)

            def proj_v_units(tg, out):
                def ones_unit(tg=tg):
                    nc.gpsimd.memset(
                        va_g[tg].rearrange("p t (h e) -> p t h e",
                                           e=DH + 1)[:, :, :, DH], 1.0)
                    xvt = xin.tile([P, kc_n, QGW], BF16, tag="xvstream",
                                   bufs=2, name=f"xvt_{tg}")
                    nc.sync.dma_start(xvt[:], xvT[tg])
                    ones_unit.xvt = xvt
                out.append(ones_unit)
                for ti in range(tpg):
                    def v_unit(tg=tg, ti=ti, holder=ones_unit):
                        xvt = holder.xvt
                        vp = ps.tile([P, QGW], F32, tag="pacc", bufs=2,
                                     name=f"vp_{tg}_{ti}")
                        if use_bias:
                            nc.tensor.matmul(vp[:, :hd],
                                             lhsT=onesb_sb[0:1, :],
                                             rhs=bv_sb[:, :], start=True,
                                             stop=False)
                        for kc in range(kc_n):
                            nc.tensor.matmul(
                                vp[:, :hd],
                                lhsT=xvt[:, kc, ti * P:(ti + 1) * P],
                                rhs=wv_sb[:, kc, :],
                                start=(kc == 0 and not use_bias),
                                stop=(kc == kc_n - 1),
                            )
                        nc.vector.tensor_copy(
                            va_g[tg][:, ti].rearrange(
                                "p (h e) -> p h e", e=DH + 1)[:, :, :DH],
                            vp[:, :hd].rearrange("p (h e) -> p h e", e=DH))
                    out.append(v_unit)

            def ff_units(qg, out, tail=False):
                for nck in range(kc_n):
                    def ff_unit(qg=qg, nck=nck, tail=tail):
                        fp = ps.tile([P, QGW], F32, tag="pacc", bufs=2,
                                     name=f"fp_{nck}_{qg}")
                        for dc in range(dc_n):
                            nc.tensor.matmul(
                                fp[:],
                                lhsT=wff_sb[:, dc, nck * P:(nck + 1) * P],
                                rhs=at_g[qg][:, dc, :],
                                start=(dc == 0),
                                stop=(dc == dc_n - 1),
                            )
                        ot = opool.tile([P, QGW], BF16, tag="otile",
                                        name=f"ot_{nck}_{qg}")
                        if use_bias:
                            nc.scalar.activation(ot[:], fp[:], AF.Identity,
                                                 bias=bffq_sb[:, nck:nck + 1])
                        elif tail:
                            # exp is done by now; use the idle scalar engine
                            nc.scalar.copy(ot[:], fp[:])
                        else:
                            nc.vector.tensor_copy(ot[:], fp[:])
                        nc.sync.dma_start(
                            outT[nck * P:(nck + 1) * P,
                                 qg * QGW:(qg + 1) * QGW], ot[:])
                    out.append(ff_unit)

            # two filler tiers: pf must drain before the next attention
            # group (projections); lf (ff) can slip to late groups.
            from collections import deque
            pf = deque()
            lf = deque()

            def run_units(n=None):
                k = (len(pf) + len(lf)) if n is None else n
                for _ in range(k):
                    if pf:
                        pf.popleft()()
                    elif lf:
                        lf.popleft()()
                    else:
                        return

            def drain_pf():
                while pf:
                    pf.popleft()()

            def attention(qg):
                kmax = (qg + 1) * tpg if variant == "causal" else tt
                for hp in range(hpc // 2):
                    dch = hp
                    h0 = 2 * hp
                    op_h = [
                        ps.tile([P, QGW], F32, tag="opacc", bufs=2,
                                name=f"op{j}_{hp}_{qg}")
                        for j in range(2)
                    ]
                    ets = {}

                    def emit_scores(kt, hp=hp, dch=dch, ets=ets, qg=qg):
                        off = (max(0, kt * P - qg * QGW)
                               if variant == "causal" else 0)
                        kg, kx = divmod(kt, tpg)
                        sp = ps.tile([P, 2, QGW], F32, tag="mmw", bufs=2,
                                     name=f"sp_{hp}_{qg}_{kt}")
                        for j in range(2):
                            r = j * DH
                            nc.tensor.matmul(
                                sp[:, j, off:],
                                lhsT=kT_g[kg][r:r + DH, dch,
                                              kx * P:(kx + 1) * P],
                                rhs=qT_g[qg][r:r + DH, dch, off:],
                                start=True,
                                stop=True,
                            )
                        if variant == "generic":
                            mb_sb = xin.tile([P, QGW], F32, tag="mstream",
                                             bufs=4,
                                             name=f"mb_{hp}_{qg}_{kt}")
                            nc.sync.dma_start(
                                mb_sb[:],
                                mbT[kt * P:(kt + 1) * P,
                                    qg * QGW:(qg + 1) * QGW])
                            for j in range(2):
                                nc.vector.tensor_add(
                                    sp[:, j, :], sp[:, j, :], mb_sb[:])
                        et = epool.tile([P, 2, QGW], BF16, tag="etile",
                                        name=f"et_{hp}_{qg}_{kt}")
                        # one ACTIVATE covers both heads even when the
                        # leading `off` columns are trimmed (3D AP)
                        nc.scalar.activation(et[:, :, off:], sp[:, :, off:],
                                             AF.Exp)
                        if variant == "causal" and kt * P - qg * QGW >= 0:
                            doff = kt * P - qg * QGW
                            for j in range(2):
                                nc.vector.tensor_mul(
                                    et[:, j, doff:doff + P],
                                    et[:, j, doff:doff + P],
                                    dmask_sb[:])
                        ets[kt] = (et, off)

                    def emit_av(kt, hp=hp, ets=ets, qg=qg, kmax=kmax,
                                op_h=op_h, h0=h0):
                        et, off = ets.pop(kt)
                        kg, kx = divmod(kt, tpg)
                        for j in range(2):
                            h = h0 + j
                            nc.tensor.matmul(
                                op_h[j][:DH + 1, off:],
                                lhsT=va_g[kg][:, kx, h * (DH + 1):
                                              (h + 1) * (DH + 1)],
                                rhs=et[:, j, off:],
                                start=(kt == 0),
                                stop=(kt == kmax - 1),
                            )

                    emit_scores(0)
                    for kt in range(1, kmax):
                        emit_scores(kt)
                        run_units(1)
                        emit_av(kt - 1)
                    emit_av(kmax - 1)
                    run_units(1)

                    # normalize: broadcast Z across partitions with a rank-1
                    # fp16 matmul, take 1/Z at full 128-lane width, and fuse
                    # the psum->sbuf cast with the multiply.
                    for j in range(2):
                        h = h0 + j
                        op = op_h[j]
                        po = (h * DH) % P
                        zrh = zp.tile([1, QGW], FP16, tag="zrh",
                                      name=f"zrh_{h}_{qg}")
                        nc.vector.tensor_copy(zrh[0:1, :], op[DH:DH + 1, :])
                        zbz = ps.tile([P, QGW], F32, tag="pacc", bufs=2,
                                      name=f"zbz_{h}_{qg}")
                        nc.tensor.matmul(
                            zbz[:],
                            lhsT=ones_sb[0:1, :],
                            rhs=zrh[0:1, :],
                            start=True, stop=True)
                        # custom-DVE ops ignore input base partitions, but
                        # this one reads a full base-0 tile (verified OK)
                        zbi = zp.tile([P, QGW], F32, tag="zbi", bufs=2,
                                      name=f"zbi_{h}_{qg}")
                        nc.vector.reciprocal_approx_fast(zbi[:], zbz[:])
                        nc.vector.tensor_mul(
                            at_g[qg][po:po + DH, dch, :],
                            op[:DH, :],
                            zbi[po:po + DH, :])

            # ---- schedule over token groups ----
            def queue_proj(tg, first=False):
                bqs = bq_sb if use_bias else None
                bks = bk_sb if use_bias else None
                if first:
                    # head-pair-0 (dc 0) projections first so attention(0)
                    # can start before pair-1's chains run (as fillers)
                    pf.append(lambda: load_w("wq", wq_sb, wqT, 0))
                    pf.append(lambda tg=tg: load_x("xq", tg, xq_g[tg], xqT))
                    pf.append(lambda: load_w("wq", wq_sb, wqT, 1))
                    proj_qk_units(tg, wq_sb, xq_g[tg], bqs, qT_g[tg], pf,
                                  dcs=(0,))
                    pf.append(lambda: load_w("wk", wk_sb, wkT))
                    pf.append(lambda tg=tg: load_x("xk", tg, xk_g[tg], xkT))
                    proj_qk_units(tg, wk_sb, xk_g[tg], bks, kT_g[tg], pf,
                                  dcs=(0,))
                    pf.append(lambda: load_w("wv", wv_sb, wvT))
                    proj_v_units(tg, pf)
                    proj_qk_units(tg, wq_sb, xq_g[tg], bqs, qT_g[tg], pf,
                                  dcs=(1,))
                    proj_qk_units(tg, wk_sb, xk_g[tg], bks, kT_g[tg], pf,
                                  dcs=(1,))
                    return
                pf.append(lambda tg=tg: load_x("xq", tg, xq_g[tg], xqT))
                proj_qk_units(tg, wq_sb, xq_g[tg], bqs, qT_g[tg], pf)
                pf.append(lambda tg=tg: load_x("xk", tg, xk_g[tg], xkT))
                proj_qk_units(tg, wk_sb, xk_g[tg], bks, kT_g[tg], pf)
                proj_v_units(tg, pf)

            if variant == "causal":
                queue_proj(0, first=True)
                # drain all but the 4 dc-1 chains; those become fillers
                # inside attention(0) and only gate head-pair 1
                run_units(len(pf) - 4)
                for tg in range(tg_n):
                    if tg + 1 < tg_n:
                        queue_proj(tg + 1)
                    lf.append(lambda: load_w("wff", wff_sb, wffT))
                    if tg > 0:
                        ff_units(tg - 1, lf)
                    attention(tg)
                    drain_pf()
                ff_units(tg_n - 1, lf, tail=True)
                run_units()
            else:
                queue_proj(0, first=True)
                for tg in range(1, tg_n):
                    queue_proj(tg)
                drain_pf()
                lf.append(lambda: load_w("wff", wff_sb, wffT))
                for qg in range(tg_n):
                    if qg > 0:
                        ff_units(qg - 1, lf)
                    attention(qg)
                ff_units(tg_n - 1, lf, tail=True)
                run_units()

    nc.compile()
    return nc


def _classify_mask(mask: np.ndarray) -> str:
    m = np.asarray(mask)[:, 0]  # [B, S, S]
    if not m.any():
        return "dense"
    s = m.shape[-1]
    causal = np.triu(np.ones((s, s), dtype=m.dtype), k=1)
    if all(np.array_equal(m[b], causal) for b in range(m.shape[0])):
        return "causal"
    return "generic"


def _bf(x):
    return np.ascontiguousarray(np.ascontiguousarray(x).astype(NPBF16))


def _tile_x(xT):
    """[d, s] -> [tg, P, kc, QGW] partition-major (8KB contiguous lines)."""
    kc = xT.shape[0] // P
    tg = xT.shape[1] // QGW
    return np.ascontiguousarray(
        xT.reshape(kc, P, tg, QGW).transpose(2, 1, 0, 3))


def _tile_w(wT):
    """[cP, m] -> [P, c, m] partition-major."""
    c = wT.shape[0] // P
    return np.ascontiguousarray(
        wT.reshape(c, P, wT.shape[1]).transpose(1, 0, 2))


def _make_in_maps(variant, query, key, value, mask, wq, bq, wk, bk, wv, bv,
                  wff, bff, use_bias):
    scale = np.float32(1.0 / np.sqrt(np.float32(DH)))
    wqTs = _bf((wq * scale).T)
    wkT = _bf(wk.T)
    wvT = _bf(wv.T)
    wffT = _bf(wff.T)

    qT = [_tile_x(_bf(query[b].T)) for b in range(B)]
    kT = [_tile_x(_bf(key[b].T)) for b in range(B)]
    vT = [_tile_x(_bf(value[b].T)) for b in range(B)]
    mbT = None
    if variant == "generic":
        mbT = [np.ascontiguousarray(mask[b, 0].T * np.float32(-1e9))
               for b in range(B)]

    dmask = np.tril(np.ones((P, P), np.float32)).T

    in_maps = []
    for c in range(NCORES):
        b, hg = c // GPB, c % GPB
        sl = slice(hg * HD, (hg + 1) * HD)
        m = {
            "xqT": qT[b], "xkT": kT[b], "xvT": vT[b],
            "wqT": _tile_w(np.ascontiguousarray(wqTs[:, sl])),
            "wkT": _tile_w(np.ascontiguousarray(wkT[:, sl])),
            "wvT": _tile_w(np.ascontiguousarray(wvT[:, sl])),
            "wffT": _tile_w(np.ascontiguousarray(wffT[sl, :])),
        }
        if use_bias:
            m["bq"] = np.ascontiguousarray((bq * scale)[sl]).astype(np.float32)
            m["bk"] = np.ascontiguousarray(bk[sl]).astype(np.float32)
            m["bv"] = _bf(bv[sl])[None, :]
            m["bffq"] = (bff / GPB).astype(np.float32)
            m["onesb"] = np.ones((1, P), NPBF16)
        if variant == "causal":
            m["dmask"] = _bf(dmask)
        if variant == "generic":
            m["mbT"] = mbT[b]
        in_maps.append(m)
    return in_maps


def kernel(**inputs) -> np.ndarray:
    query = np.ascontiguousarray(inputs["query"], dtype=np.float32)
    key = np.ascontiguousarray(inputs["key"], dtype=np.float32)
    value = np.ascontiguousarray(inputs["value"], dtype=np.float32)
    mask = np.asarray(inputs["mask"], dtype=np.float32)
    wq = np.asarray(inputs["wq"], np.float32)
    bq = np.asarray(inputs["bq"], np.float32)
    wk = np.asarray(inputs["wk"], np.float32)
    bk = np.asarray(inputs["bk"], np.float32)
    wv = np.asarray(inputs["wv"], np.float32)
    bv = np.asarray(inputs["bv"], np.float32)
    wff = np.asarray(inputs["wff"], np.float32)
    bff = np.asarray(inputs["bff"], np.float32)

    variant = _classify_mask(mask)
    use_bias = bool(bq.any() or bk.any() or bv.any() or bff.any())
    pkey = (variant, use_bias)
    if pkey not in _PROG_CACHE:
        _PROG_CACHE[pkey] = build_program(variant, use_bias)
    nc = _PROG_CACHE[pkey]

    in_maps = _make_in_maps(variant, query, key, value, mask, wq, bq, wk, bk,
                            wv, bv, wff, bff, use_bias)
    res = run_bass_kernel_spmd(nc, in_maps, core_ids=list(range(NCORES)))
    out = np.empty((B, S, D), np.float32)
    for b in range(B):
        acc = res.results[b * GPB]["outT"].astype(np.float32)
        for g in range(1, GPB):
            acc = acc + res.results[b * GPB + g]["outT"].astype(np.float32)
        out[b] = acc.T
    return out


if __name__ == "__main__":
    import reference

    inputs = {k: np.asarray(v) for k, v in reference.setup_inputs().items()}
    out = kernel(**inputs)
    print("kernel out:", out.shape, out.dtype)


# revision 22
# speedup vs baseline: 1.0110x; 1.0040x over previous
"""Trainium2 Bass kernel for nn_MultiHeadAttention (B=2, S=2048, D=1024, H=16).

Sharding: 8 cores = 2 batches x 4 head-groups (4 heads per core, tensor
parallel over heads). Each core computes, for its batch b and its 4 heads:
  QT/KT = (x @ W.T).T projections in transposed layout [256, 2048]
  V     = value @ wv.T in normal layout, augmented with a ones column (Z trick)
  E^T   = exp(scoresT) tiles [k,q] directly from matmul (no max subtraction;
          scores are O(1) for this module so exp is safe, and masked entries
          use a multiplicative 0/1 mask so they are exactly 0)
  outT  = V_aug.T @ E^T accumulated over k tiles -> row 64 carries Z = sum(E)
  ffT   = wff_rows-partial @ (attn_outT * 1/Z) + bff/4   as [1024, 2048]
Host sums the 4 partial ffT per batch and transposes back.

Performance structure:
  - A dummy-matmul warmup chain at t=0 keeps the PE HAM clock-gate at
    2.4 GHz from ~4us (otherwise the first ~50us run at 1.2 GHz).
  - Score matmuls for a head PAIR are packed into PE row-tiles (0,0) and
    (64,0) (contraction is DH=64), doubling score throughput.
  - The softmax Z is broadcast across partitions with a rank-1 fp16 PE
    matmul (no DRAM round-trip); 1/Z runs at full 128-lane DVE width and
    the normalize-multiply is fused with the psum->sbuf cast.
  - AV matmuls trim fully-masked leading columns (causal).
  - x loads stream per 512-token group (half-K granularity) so the first
    projection starts ~2us in; output is written in bf16.
  - Projections of group g+1 and ff of group g-1 are queued as filler
    units between attention matmuls to cover the exp latency; ff units
    are deferred (lazy queue) to the late, filler-poor attention groups.
"""

import sys

sys.path.insert(0, "/opt/trn_rl_repo")

import ml_dtypes
import numpy as np

import concourse.bass as bass
import concourse.mybir as mybir
import concourse.tile as tile
from concourse import bacc
from concourse.bass_utils import run_bass_kernel_spmd

P = 128
B, S, D, H = 2, 2048, 1024, 16
DH = D // H  # 64
NCORES = 8
GPB = NCORES // B  # cores (head groups) per batch = 4
HPC = H // GPB  # heads per core = 4
HD = HPC * DH  # projected cols per core = 256
F32 = mybir.dt.float32
F32R = mybir.dt.float32r
FP16 = mybir.dt.float16
BF16 = mybir.dt.bfloat16
QGW = 512  # q-group width (psum free dim)
AF = mybir.ActivationFunctionType
NPBF16 = ml_dtypes.bfloat16
NWARM = 9   # warmup matmuls to engage the HAM clock un-throttle

_PROG_CACHE: dict = {}


def build_program(variant: str, use_bias: bool, s=S, d=D, hpc=HPC,
                  n_devices=NCORES):
    """variant: 'causal' | 'dense' | 'generic'. Returns compiled Bacc."""
    assert variant in ("causal", "dense", "generic")
    kc_n = d // P           # contraction chunks over model dim
    tt = s // P             # token tiles
    hd = hpc * DH           # per-core projected width
    dc_n = hd // P          # dout chunks for QT/KT (and hd chunks for ff)
    tg_n = s // QGW         # token/q groups
    tpg = QGW // P          # token tiles per group (4)
    khalf = kc_n // 2

    nc = bacc.Bacc("TRN2", target_bir_lowering=False, debug=False,
                   num_devices=n_devices)

    def din(name, shape, dt=BF16):
        return nc.dram_tensor(name, list(shape), dt, kind="ExternalInput").ap()

    # activations/weights arrive pre-tiled partition-major from the host
    # ([...,P, c, m] contiguous) so every DMA is few large descriptor lines
    xqT = din("xqT", (tg_n, P, kc_n, QGW))
    xkT = din("xkT", (tg_n, P, kc_n, QGW))
    xvT = din("xvT", (tg_n, P, kc_n, QGW))
    wqT = din("wqT", (P, kc_n, hd))   # pre-scaled by 1/sqrt(DH) on host
    wkT = din("wkT", (P, kc_n, hd))
    wvT = din("wvT", (P, kc_n, hd))
    wffT = din("wffT", (P, dc_n, d))
    if use_bias:
        bq = din("bq", (hd,), F32)   # pre-scaled by 1/sqrt(DH) on host
        bk = din("bk", (hd,), F32)
        bv = din("bv", (1, hd))
        bffq = din("bffq", (d,), F32)    # bff / GPB
        onesb = din("onesb", (1, P))
    if variant == "causal":
        dmask = din("dmask", (P, P))  # [k, q]: 1 if k <= q else 0
    if variant == "generic":
        mbT = din("mbT", (s, s), F32)  # mask[b,0].T * -1e9, [k, q] layout
    outT = nc.dram_tensor("outT", [d, s], BF16, kind="ExternalOutput").ap()

    with tile.TileContext(nc) as tc:
        with (
            nc.allow_low_precision(reason="bf16 matmul chain; psum stays fp32"),
            tc.tile_pool(name="consts", bufs=1) as consts,
            tc.tile_pool(name="xin", bufs=1) as xin,
            tc.tile_pool(name="acts", bufs=1) as acts,
            tc.tile_pool(name="epool", bufs=8) as epool,
            tc.tile_pool(name="opool", bufs=4) as opool,
            tc.tile_pool(name="zp", bufs=4) as zp,
            tc.tile_pool(name="ps", bufs=1, space="PSUM") as ps,
        ):
            # ---- warmup: engage the PE clock un-throttle immediately.
            # Must be full-K matmuls: K=1 ones do not register as PE
            # activity for the HAM monitor (measured: stayed cold to 37us).
            warm = consts.tile([P, P + QGW], BF16, tag="warm")
            nc.gpsimd.memset(warm[:], 0.125)
            wmp = ps.tile([P, QGW], F32, tag="opacc", bufs=2, name="wmp")
            for i in range(NWARM):
                nc.tensor.matmul(wmp[:], lhsT=warm[:, :P],
                                 rhs=warm[:, P:],
                                 start=(i == 0), stop=(i == NWARM - 1))
            warm_out = consts.tile([1, QGW], BF16, tag="warmout")
            nc.vector.tensor_copy(warm_out[0:1, :], wmp[0:1, :])
            # preload the scalar engine's exp table during the warmup so the
            # first real exp doesn't pay the ~2.7us ACT_TABLE_LOAD
            expdummy = consts.tile([1, 8], BF16, tag="expdummy")
            nc.scalar.activation(expdummy[0:1, :], warm[0:1, :8], AF.Exp)

            # ones row for the rank-1 1/Z partition-broadcast
            ones_sb = consts.tile([1, P], FP16, tag="ones")
            nc.gpsimd.memset(ones_sb[:], 1.0)

            # ---- constant / weight tiles ----
            wq_sb = consts.tile([P, kc_n, hd], BF16, tag="wq")
            wk_sb = consts.tile([P, kc_n, hd], BF16, tag="wk")
            wv_sb = consts.tile([P, kc_n, hd], BF16, tag="wv")
            wff_sb = consts.tile([P, dc_n, d], BF16, tag="wff")
            _loaded = set()

            def load_w(name, sb, dram, half=None):
                key = (name, half)
                if key in _loaded:
                    return
                _loaded.add(key)
                if half is None:
                    nc.sync.dma_start(sb[:], dram)
                else:
                    k0 = half * khalf
                    nc.sync.dma_start(sb[:, k0:k0 + khalf, :],
                                      dram[:, k0:k0 + khalf, :])
            if use_bias:
                bq_sb = consts.tile([P, dc_n], F32, tag="bq")
                bk_sb = consts.tile([P, dc_n], F32, tag="bk")
                nc.sync.dma_start(bq_sb[:], bq.rearrange("(c p) -> p c", p=P))
                nc.sync.dma_start(bk_sb[:], bk.rearrange("(c p) -> p c", p=P))
                bv_sb = consts.tile([1, hd], BF16, tag="bv")
                nc.sync.dma_start(bv_sb[:], bv[:])
                bffq_sb = consts.tile([P, kc_n], F32, tag="bffq")
                nc.sync.dma_start(bffq_sb[:],
                                  bffq.rearrange("(c p) -> p c", p=P))
                onesb_sb = consts.tile([1, P], BF16, tag="onesb")
                nc.sync.dma_start(onesb_sb[:], onesb[:])
            if variant == "causal":
                dmask_sb = consts.tile([P, P], BF16, tag="dmask")
                nc.sync.dma_start(dmask_sb[:], dmask[:])

            # per-group activation tiles
            xq_g = [acts.tile([P, kc_n, QGW], BF16, tag=f"xq{g}",
                              name=f"xq_{g}") for g in range(tg_n)]
            xk_g = [acts.tile([P, kc_n, QGW], BF16, tag=f"xk{g}",
                              name=f"xk_{g}") for g in range(tg_n)]
            qT_g = [acts.tile([P, dc_n, QGW], BF16, tag=f"qT{g}",
                              name=f"qT_{g}") for g in range(tg_n)]
            kT_g = [acts.tile([P, dc_n, QGW], BF16, tag=f"kT{g}",
                              name=f"kT_{g}") for g in range(tg_n)]
            va_g = [acts.tile([P, tpg, hpc * (DH + 1)], BF16, tag=f"va{g}",
                              name=f"va_{g}") for g in range(tg_n)]
            at_g = [acts.tile([P, dc_n, QGW], BF16, tag=f"at{g}",
                              name=f"at_{g}") for g in range(tg_n)]

            _xdma_done = set()

            def load_x(name, g, x_sb, x_dram):
                """Load activations for one token group. Group 0 is split
                into half-K DMAs so the first projection chain starts
                sooner; later groups use one DMA to cut SP issue cost."""
                if (name, g) in _xdma_done:
                    return
                _xdma_done.add((name, g))
                src = x_dram[g]
                if g == 0:
                    for half in range(2):
                        k0 = half * khalf
                        nc.sync.dma_start(x_sb[:, k0:k0 + khalf, :],
                                          src[:, k0:k0 + khalf, :])
                else:
                    nc.sync.dma_start(x_sb[:], src)

            def proj_qk_units(tg, w_sb, x_sb, b_sb, dest, out, dcs=None):
                """Append filler units: 2 half-chains per dc."""
                cell = {}
                for dc in (range(dc_n) if dcs is None else dcs):
                    for half in range(2):
                        def chain(tg=tg, dc=dc, half=half, w_sb=w_sb,
                                  x_sb=x_sb, b_sb=b_sb, dest=dest):
                            if half == 0:
                                cell[dc] = ps.tile([P, QGW], F32, tag="pacc",
                                                   bufs=2,
                                                   name=f"pp_{tg}_{dc}")
                            pp = cell[dc]
                            k0 = half * khalf
                            for kc in range(k0, k0 + khalf):
                                nc.tensor.matmul(
                                    pp[:],
                                    lhsT=w_sb[:, kc, dc * P:(dc + 1) * P],
                                    rhs=x_sb[:, kc, :],
                                    start=(kc == 0),
                                    stop=(kc == kc_n - 1),
                                )
                            if half == 1:
                                if use_bias:
                                    nc.scalar.activation(
                                        dest[:, dc, :], pp[:], AF.Identity,
                                        bias=b_sb[:, dc:dc + 1])
                                else:
                                    nc.vector.tensor_copy(dest[:, dc, :],
                                                          pp[:])
                        out.append(chain)

            _xvt = {}

            def load_xv(tg):
                if tg in _xvt:
                    return
                xvt = xin.tile([P, kc_n, QGW], BF16, tag="xvstream",
                               bufs=2, name=f"xvt_{tg}")
                nc.sync.dma_start(xvt[:], xvT[tg])
                _xvt[tg] = xvt

            def proj_v_units(tg, out):
                def ones_unit(tg=tg):
                    load_xv(tg)
                    nc.gpsimd.memset(
                        va_g[tg].rearrange("p t (h e) -> p t h e",
                                           e=DH + 1)[:, :, :, DH], 1.0)
                out.append(ones_unit)
                for ti in range(tpg):
                    def v_unit(tg=tg, ti=ti):
                        xvt = _xvt[tg]
                        vp = ps.tile([P, QGW], F32, tag="pacc", bufs=2,
                                     name=f"vp_{tg}_{ti}")
                        if use_bias:
                            nc.tensor.matmul(vp[:, :hd],
                                             lhsT=onesb_sb[0:1, :],
                                             rhs=bv_sb[:, :], start=True,
                                             stop=False)
                        for kc in range(kc_n):
                            nc.tensor.matmul(
                                vp[:, :hd],
                                lhsT=xvt[:, kc, ti * P:(ti + 1) * P],
                                rhs=wv_sb[:, kc, :],
                                start=(kc == 0 and not use_bias),
                                stop=(kc == kc_n - 1),
                            )
                        nc.vector.tensor_copy(
                            va_g[tg][:, ti].rearrange(
                                "p (h e) -> p h e", e=DH + 1)[:, :, :DH],
                            vp[:, :hd].rearrange("p (h e) -> p h e", e=DH))
                    out.append(v_unit)

            def ff_units(qg, out, tail=False):
                for nck in range(kc_n):
                    def ff_unit(qg=qg, nck=nck, tail=tail):
                        fp = ps.tile([P, QGW], F32, tag="pacc", bufs=2,
                                     name=f"fp_{nck}_{qg}")
                        for dc in range(dc_n):
                            nc.tensor.matmul(
                                fp[:],
                                lhsT=wff_sb[:, dc, nck * P:(nck + 1) * P],
                                rhs=at_g[qg][:, dc, :],
                                start=(dc == 0),
                                stop=(dc == dc_n - 1),
                            )
                        ot = opool.tile([P, QGW], BF16, tag="otile",
                                        name=f"ot_{nck}_{qg}")
                        if use_bias:
                            nc.scalar.activation(ot[:], fp[:], AF.Identity,
                                                 bias=bffq_sb[:, nck:nck + 1])
                        elif tail:
                            # exp is done by now; use the idle scalar engine
                            nc.scalar.copy(ot[:], fp[:])
                        else:
                            nc.vector.tensor_copy(ot[:], fp[:])
                        nc.sync.dma_start(
                            outT[nck * P:(nck + 1) * P,
                                 qg * QGW:(qg + 1) * QGW], ot[:])
                    out.append(ff_unit)

            # two filler tiers: pf must drain before the next attention
            # group (projections); lf (ff) can slip to late groups.
            from collections import deque
            pf = deque()
            lf = deque()

            def run_units(n=None):
                k = (len(pf) + len(lf)) if n is None else n
                for _ in range(k):
                    if pf:
                        pf.popleft()()
                    elif lf:
                        lf.popleft()()
                    else:
                        return

            def drain_pf():
                while pf:
                    pf.popleft()()

            def attention(qg):
                kmax = (qg + 1) * tpg if variant == "causal" else tt
                for hp in range(hpc // 2):
                    dch = hp
                    h0 = 2 * hp
                    op_h = [
                        ps.tile([P, QGW], F32, tag="opacc", bufs=2,
                                name=f"op{j}_{hp}_{qg}")
                        for j in range(2)
                    ]
                    ets = {}

                    def emit_scores(kt, hp=hp, dch=dch, ets=ets, qg=qg):
                        off = (max(0, kt * P - qg * QGW)
                               if variant == "causal" else 0)
                        kg, kx = divmod(kt, tpg)
                        sp = ps.tile([P, 2, QGW], F32, tag="mmw", bufs=2,
                                     name=f"sp_{hp}_{qg}_{kt}")
                        for j in range(2):
                            r = j * DH
                            nc.tensor.matmul(
                                sp[:, j, off:],
                                lhsT=kT_g[kg][r:r + DH, dch,
                                              kx * P:(kx + 1) * P],
                                rhs=qT_g[qg][r:r + DH, dch, off:],
                                start=True,
                                stop=True,
                            )
                        if variant == "generic":
                            mb_sb = xin.tile([P, QGW], F32, tag="mstream",
                                             bufs=4,
                                             name=f"mb_{hp}_{qg}_{kt}")
                            nc.sync.dma_start(
                                mb_sb[:],
                                mbT[kt * P:(kt + 1) * P,
                                    qg * QGW:(qg + 1) * QGW])
                            for j in range(2):
                                nc.vector.tensor_add(
                                    sp[:, j, :], sp[:, j, :], mb_sb[:])
                        et = epool.tile([P, 2, QGW], BF16, tag="etile",
                                        name=f"et_{hp}_{qg}_{kt}")
                        # one ACTIVATE covers both heads even when the
                        # leading `off` columns are trimmed (3D AP)
                        nc.scalar.activation(et[:, :, off:], sp[:, :, off:],
                                             AF.Exp)
                        if variant == "causal" and kt * P - qg * QGW >= 0:
                            doff = kt * P - qg * QGW
                            for j in range(2):
                                nc.vector.tensor_mul(
                                    et[:, j, doff:doff + P],
                                    et[:, j, doff:doff + P],
                                    dmask_sb[:])
                        ets[kt] = (et, off)

                    def emit_av(kt, hp=hp, ets=ets, qg=qg, kmax=kmax,
                                op_h=op_h, h0=h0):
                        et, off = ets.pop(kt)
                        kg, kx = divmod(kt, tpg)
                        for j in range(2):
                            h = h0 + j
                            nc.tensor.matmul(
                                op_h[j][:DH + 1, off:],
                                lhsT=va_g[kg][:, kx, h * (DH + 1):
                                              (h + 1) * (DH + 1)],
                                rhs=et[:, j, off:],
                                start=(kt == 0),
                                stop=(kt == kmax - 1),
                            )

                    emit_scores(0)
                    for kt in range(1, kmax):
                        emit_scores(kt)
                        run_units(1)
                        emit_av(kt - 1)
                    emit_av(kmax - 1)
                    run_units(1)

                    # normalize: broadcast Z across partitions with a rank-1
                    # fp16 matmul, take 1/Z at full 128-lane width, and fuse
                    # the psum->sbuf cast with the multiply.
                    for j in range(2):
                        h = h0 + j
                        op = op_h[j]
                        po = (h * DH) % P
                        zrh = zp.tile([1, QGW], FP16, tag="zrh",
                                      name=f"zrh_{h}_{qg}")
                        nc.vector.tensor_copy(zrh[0:1, :], op[DH:DH + 1, :])
                        zbz = ps.tile([P, QGW], F32, tag="pacc", bufs=2,
                                      name=f"zbz_{h}_{qg}")
                        nc.tensor.matmul(
                            zbz[:],
                            lhsT=ones_sb[0:1, :],
                            rhs=zrh[0:1, :],
                            start=True, stop=True)
                        # custom-DVE ops ignore input base partitions, but
                        # this one reads a full base-0 tile (verified OK)
                        zbi = zp.tile([P, QGW], F32, tag="zbi", bufs=2,
                                      name=f"zbi_{h}_{qg}")
                        nc.vector.reciprocal_approx_fast(zbi[:], zbz[:])
                        nc.vector.tensor_mul(
                            at_g[qg][po:po + DH, dch, :],
                            op[:DH, :],
                            zbi[po:po + DH, :])

            # ---- schedule over token groups ----
            def queue_proj(tg, first=False):
                bqs = bq_sb if use_bias else None
                bks = bk_sb if use_bias else None
                if first:
                    # head-pair-0 (dc 0) projections first so attention(0)
                    # can start before pair-1's chains run (as fillers).
                    # DMA issue order matters: transfers drain the queue in
                    # order, so the v-path (wv, xv) goes first.
                    pf.append(lambda: load_w("wq", wq_sb, wqT, 0))
                    pf.append(lambda: load_w("wq", wq_sb, wqT, 1))
                    pf.append(lambda: load_w("wv", wv_sb, wvT))
                    pf.append(lambda tg=tg: load_xv(tg))
                    pf.append(lambda tg=tg: load_x("xq", tg, xq_g[tg], xqT))
                    proj_qk_units(tg, wq_sb, xq_g[tg], bqs, qT_g[tg], pf,
                                  dcs=(0,))
                    pf.append(lambda: load_w("wk", wk_sb, wkT))
                    pf.append(lambda tg=tg: load_x("xk", tg, xk_g[tg], xkT))
                    proj_qk_units(tg, wk_sb, xk_g[tg], bks, kT_g[tg], pf,
                                  dcs=(0,))
                    proj_v_units(tg, pf)
                    proj_qk_units(tg, wq_sb, xq_g[tg], bqs, qT_g[tg], pf,
                                  dcs=(1,))
                    proj_qk_units(tg, wk_sb, xk_g[tg], bks, kT_g[tg], pf,
                                  dcs=(1,))
                    return
                pf.append(lambda tg=tg: load_x("xq", tg, xq_g[tg], xqT))
                proj_qk_units(tg, wq_sb, xq_g[tg], bqs, qT_g[tg], pf)
                pf.append(lambda tg=tg: load_x("xk", tg, xk_g[tg], xkT))
                proj_qk_units(tg, wk_sb, xk_g[tg], bks, kT_g[tg], pf)
                proj_v_units(tg, pf)

            if variant == "causal":
                queue_proj(0, first=True)
                # drain all but the 4 dc-1 chains; those become fillers
                # inside attention(0) and only gate head-pair 1
                run_units(len(pf) - 4)
                for tg in range(tg_n):
                    if tg + 1 < tg_n:
                        queue_proj(tg + 1)
                    lf.append(lambda: load_w("wff", wff_sb, wffT))
                    if tg > 0:
                        ff_units(tg - 1, lf)
                    attention(tg)
                    drain_pf()
                ff_units(tg_n - 1, lf, tail=True)
                run_units()
            else:
                queue_proj(0, first=True)
                for tg in range(1, tg_n):
                    queue_proj(tg)
                drain_pf()
                lf.append(lambda: load_w("wff", wff_sb, wffT))
                for qg in range(tg_n):
                    if qg > 0:
                        ff_units(qg - 1, lf)
                    attention(qg)
                ff_units(tg_n - 1, lf, tail=True)
                run_units()

    nc.compile()
    return nc


def _classify_mask(mask: np.ndarray) -> str:
    m = np.asarray(mask)[:, 0]  # [B, S, S]
    if not m.any():
        return "dense"
    s = m.shape[-1]
    causal = np.triu(np.ones((s, s), dtype=m.dtype), k=1)
    if all(np.array_equal(m[b], causal) for b in range(m.shape[0])):
        return "causal"
    return "generic"


def _bf(x):
    return np.ascontiguousarray(np.ascontiguousarray(x).astype(NPBF16))


def _tile_x(xT):
    """[d, s] -> [tg, P, kc, QGW] partition-major (8KB contiguous lines)."""
    kc = xT.shape[0] // P
    tg = xT.shape[1] // QGW
    return np.ascontiguousarray(
        xT.reshape(kc, P, tg, QGW).transpose(2, 1, 0, 3))


def _tile_w(wT):
    """[cP, m] -> [P, c, m] partition-major."""
    c = wT.shape[0] // P
    return np.ascontiguousarray(
        wT.reshape(c, P, wT.shape[1]).transpose(1, 0, 2))


def _make_in_maps(variant, query, key, value, mask, wq, bq, wk, bk, wv, bv,
                  wff, bff, use_bias):
    scale = np.float32(1.0 / np.sqrt(np.float32(DH)))
    wqTs = _bf((wq * scale).T)
    wkT = _bf(wk.T)
    wvT = _bf(wv.T)
    wffT = _bf(wff.T)

    qT = [_tile_x(_bf(query[b].T)) for b in range(B)]
    kT = [_tile_x(_bf(key[b].T)) for b in range(B)]
    vT = [_tile_x(_bf(value[b].T)) for b in range(B)]
    mbT = None
    if variant == "generic":
        mbT = [np.ascontiguousarray(mask[b, 0].T * np.float32(-1e9))
               for b in range(B)]

    dmask = np.tril(np.ones((P, P), np.float32)).T

    in_maps = []
    for c in range(NCORES):
        b, hg = c // GPB, c % GPB
        sl = slice(hg * HD, (hg + 1) * HD)
        m = {
            "xqT": qT[b], "xkT": kT[b], "xvT": vT[b],
            "wqT": _tile_w(np.ascontiguousarray(wqTs[:, sl])),
            "wkT": _tile_w(np.ascontiguousarray(wkT[:, sl])),
            "wvT": _tile_w(np.ascontiguousarray(wvT[:, sl])),
            "wffT": _tile_w(np.ascontiguousarray(wffT[sl, :])),
        }
        if use_bias:
            m["bq"] = np.ascontiguousarray((bq * scale)[sl]).astype(np.float32)
            m["bk"] = np.ascontiguousarray(bk[sl]).astype(np.float32)
            m["bv"] = _bf(bv[sl])[None, :]
            m["bffq"] = (bff / GPB).astype(np.float32)
            m["onesb"] = np.ones((1, P), NPBF16)
        if variant == "causal":
            m["dmask"] = _bf(dmask)
        if variant == "generic":
            m["mbT"] = mbT[b]
        in_maps.append(m)
    return in_maps


def kernel(**inputs) -> np.ndarray:
    query = np.ascontiguousarray(inputs["query"], dtype=np.float32)
    key = np.ascontiguousarray(inputs["key"], dtype=np.float32)
    value = np.ascontiguousarray(inputs["value"], dtype=np.float32)
    mask = np.asarray(inputs["mask"], dtype=np.float32)
    wq = np.asarray(inputs["wq"], np.float32)
    bq = np.asarray(inputs["bq"], np.float32)
    wk = np.asarray(inputs["wk"], np.float32)
    bk = np.asarray(inputs["bk"], np.float32)
    wv = np.asarray(inputs["wv"], np.float32)
    bv = np.asarray(inputs["bv"], np.float32)
    wff = np.asarray(inputs["wff"], np.float32)
    bff = np.asarray(inputs["bff"], np.float32)

    variant = _classify_mask(mask)
    use_bias = bool(bq.any() or bk.any() or bv.any() or bff.any())
    pkey = (variant, use_bias)
    if pkey not in _PROG_CACHE:
        _PROG_CACHE[pkey] = build_program(variant, use_bias)
    nc = _PROG_CACHE[pkey]

    in_maps = _make_in_maps(variant, query, key, value, mask, wq, bq, wk, bk,
                            wv, bv, wff, bff, use_bias)
    res = run_bass_kernel_spmd(nc, in_maps, core_ids=list(range(NCORES)))
    out = np.empty((B, S, D), np.float32)
    for b in range(B):
        acc = res.results[b * GPB]["outT"].astype(np.float32)
        for g in range(1, GPB):
            acc = acc + res.results[b * GPB + g]["outT"].astype(np.float32)
        out[b] = acc.T
    return out


if __name__ == "__main__":
    import reference

    inputs = {k: np.asarray(v) for k, v in reference.setup_inputs().items()}
    out = kernel(**inputs)
    print("kernel out:", out.shape, out.dtype)


# revision 23
# speedup vs baseline: 1.0185x; 1.0075x over previous
"""Trainium2 Bass kernel for nn_MultiHeadAttention (B=2, S=2048, D=1024, H=16).

Sharding: 8 cores = 2 batches x 4 head-groups (4 heads per core, tensor
parallel over heads). Each core computes, for its batch b and its 4 heads:
  QT/KT = (x @ W.T).T projections in transposed layout [256, 2048]
  V     = value @ wv.T in normal layout, augmented with a ones column (Z trick)
  E^T   = exp(scoresT) tiles [k,q] directly from matmul (no max subtraction;
          scores are O(1) for this module so exp is safe, and masked entries
          use a multiplicative 0/1 mask so they are exactly 0)
  outT  = V_aug.T @ E^T accumulated over k tiles -> row 64 carries Z = sum(E)
  ffT   = wff_rows-partial @ (attn_outT * 1/Z) + bff/4   as [1024, 2048]
Host sums the 4 partial ffT per batch and transposes back.

Performance structure:
  - A dummy-matmul warmup chain at t=0 keeps the PE HAM clock-gate at
    2.4 GHz from ~4us (otherwise the first ~50us run at 1.2 GHz).
  - Score matmuls for a head PAIR are packed into PE row-tiles (0,0) and
    (64,0) (contraction is DH=64), doubling score throughput.
  - The softmax Z is broadcast across partitions with a rank-1 fp16 PE
    matmul (no DRAM round-trip); 1/Z runs at full 128-lane DVE width and
    the normalize-multiply is fused with the psum->sbuf cast.
  - AV matmuls trim fully-masked leading columns (causal).
  - x loads stream per 512-token group (half-K granularity) so the first
    projection starts ~2us in; output is written in bf16.
  - Projections of group g+1 and ff of group g-1 are queued as filler
    units between attention matmuls to cover the exp latency; ff units
    are deferred (lazy queue) to the late, filler-poor attention groups.
"""

import sys

sys.path.insert(0, "/opt/trn_rl_repo")

import ml_dtypes
import numpy as np

import concourse.bass as bass
import concourse.mybir as mybir
import concourse.tile as tile
from concourse import bacc
from concourse.bass_utils import run_bass_kernel_spmd

P = 128
B, S, D, H = 2, 2048, 1024, 16
DH = D // H  # 64
NCORES = 8
GPB = NCORES // B  # cores (head groups) per batch = 4
HPC = H // GPB  # heads per core = 4
HD = HPC * DH  # projected cols per core = 256
F32 = mybir.dt.float32
F32R = mybir.dt.float32r
FP16 = mybir.dt.float16
BF16 = mybir.dt.bfloat16
QGW = 512  # q-group width (psum free dim)
AF = mybir.ActivationFunctionType
NPBF16 = ml_dtypes.bfloat16
NWARM = 12  # warmup matmuls to engage the HAM clock un-throttle

_PROG_CACHE: dict = {}


def build_program(variant: str, use_bias: bool, s=S, d=D, hpc=HPC,
                  n_devices=NCORES):
    """variant: 'causal' | 'dense' | 'generic'. Returns compiled Bacc."""
    assert variant in ("causal", "dense", "generic")
    kc_n = d // P           # contraction chunks over model dim
    tt = s // P             # token tiles
    hd = hpc * DH           # per-core projected width
    dc_n = hd // P          # dout chunks for QT/KT (and hd chunks for ff)
    tg_n = s // QGW         # token/q groups
    tpg = QGW // P          # token tiles per group (4)
    khalf = kc_n // 2

    nc = bacc.Bacc("TRN2", target_bir_lowering=False, debug=False,
                   num_devices=n_devices)

    def din(name, shape, dt=BF16):
        return nc.dram_tensor(name, list(shape), dt, kind="ExternalInput").ap()

    # activations/weights arrive pre-tiled partition-major from the host
    # ([...,P, c, m] contiguous) so every DMA is few large descriptor lines
    xqT = din("xqT", (tg_n, P, kc_n, QGW))
    xkT = din("xkT", (tg_n, P, kc_n, QGW))
    xvT = din("xvT", (tg_n, P, kc_n, QGW))
    wqT = din("wqT", (P, kc_n, hd))   # pre-scaled by 1/sqrt(DH) on host
    wkT = din("wkT", (P, kc_n, hd))
    wvT = din("wvT", (P, kc_n, hd))
    wffT = din("wffT", (P, dc_n, d))
    if use_bias:
        bq = din("bq", (hd,), F32)   # pre-scaled by 1/sqrt(DH) on host
        bk = din("bk", (hd,), F32)
        bv = din("bv", (1, hd))
        bffq = din("bffq", (d,), F32)    # bff / GPB
        onesb = din("onesb", (1, P))
    if variant == "causal":
        dmask = din("dmask", (P, P))  # [k, q]: 1 if k <= q else 0
    if variant == "generic":
        mbT = din("mbT", (s, s), F32)  # mask[b,0].T * -1e9, [k, q] layout
    outT = nc.dram_tensor("outT", [d, s], BF16, kind="ExternalOutput").ap()

    with tile.TileContext(nc) as tc:
        with (
            nc.allow_low_precision(reason="bf16 matmul chain; psum stays fp32"),
            tc.tile_pool(name="consts", bufs=1) as consts,
            tc.tile_pool(name="xin", bufs=1) as xin,
            tc.tile_pool(name="acts", bufs=1) as acts,
            tc.tile_pool(name="epool", bufs=8) as epool,
            tc.tile_pool(name="opool", bufs=4) as opool,
            tc.tile_pool(name="zp", bufs=4) as zp,
            tc.tile_pool(name="ps", bufs=1, space="PSUM") as ps,
        ):
            # ---- warmup: engage the PE clock un-throttle immediately.
            # Must be full-K matmuls: K=1 ones do not register as PE
            # activity for the HAM monitor (measured: stayed cold to 37us).
            warm = consts.tile([P, P + QGW], BF16, tag="warm")
            nc.gpsimd.memset(warm[:], 0.125)
            wmp = ps.tile([P, QGW], F32, tag="opacc", bufs=2, name="wmp")
            for i in range(NWARM):
                nc.tensor.matmul(wmp[:], lhsT=warm[:, :P],
                                 rhs=warm[:, P:],
                                 start=(i == 0), stop=(i == NWARM - 1))
            warm_out = consts.tile([1, QGW], BF16, tag="warmout")
            nc.vector.tensor_copy(warm_out[0:1, :], wmp[0:1, :])
            # preload the scalar engine's exp table during the warmup so the
            # first real exp doesn't pay the ~2.7us ACT_TABLE_LOAD
            expdummy = consts.tile([1, 8], BF16, tag="expdummy")
            nc.scalar.activation(expdummy[0:1, :], warm[0:1, :8], AF.Exp)

            # ones row for the rank-1 1/Z partition-broadcast
            ones_sb = consts.tile([1, P], FP16, tag="ones")
            nc.gpsimd.memset(ones_sb[:], 1.0)

            # ---- constant / weight tiles ----
            wq_sb = consts.tile([P, kc_n, hd], BF16, tag="wq")
            wk_sb = consts.tile([P, kc_n, hd], BF16, tag="wk")
            wv_sb = consts.tile([P, kc_n, hd], BF16, tag="wv")
            wff_sb = consts.tile([P, dc_n, d], BF16, tag="wff")
            _loaded = set()

            def load_w(name, sb, dram, half=None):
                key = (name, half)
                if key in _loaded:
                    return
                _loaded.add(key)
                if half is None:
                    nc.sync.dma_start(sb[:], dram)
                else:
                    k0 = half * khalf
                    nc.sync.dma_start(sb[:, k0:k0 + khalf, :],
                                      dram[:, k0:k0 + khalf, :])
            if use_bias:
                bq_sb = consts.tile([P, dc_n], F32, tag="bq")
                bk_sb = consts.tile([P, dc_n], F32, tag="bk")
                nc.sync.dma_start(bq_sb[:], bq.rearrange("(c p) -> p c", p=P))
                nc.sync.dma_start(bk_sb[:], bk.rearrange("(c p) -> p c", p=P))
                bv_sb = consts.tile([1, hd], BF16, tag="bv")
                nc.sync.dma_start(bv_sb[:], bv[:])
                bffq_sb = consts.tile([P, kc_n], F32, tag="bffq")
                nc.sync.dma_start(bffq_sb[:],
                                  bffq.rearrange("(c p) -> p c", p=P))
                onesb_sb = consts.tile([1, P], BF16, tag="onesb")
                nc.sync.dma_start(onesb_sb[:], onesb[:])
            if variant == "causal":
                dmask_sb = consts.tile([P, P], BF16, tag="dmask")
                nc.sync.dma_start(dmask_sb[:], dmask[:])

            # per-group activation tiles
            xq_g = [acts.tile([P, kc_n, QGW], BF16, tag=f"xq{g}",
                              name=f"xq_{g}") for g in range(tg_n)]
            xk_g = [acts.tile([P, kc_n, QGW], BF16, tag=f"xk{g}",
                              name=f"xk_{g}") for g in range(tg_n)]
            qT_g = [acts.tile([P, dc_n, QGW], BF16, tag=f"qT{g}",
                              name=f"qT_{g}") for g in range(tg_n)]
            kT_g = [acts.tile([P, dc_n, QGW], BF16, tag=f"kT{g}",
                              name=f"kT_{g}") for g in range(tg_n)]
            va_g = [acts.tile([P, tpg, hpc * (DH + 1)], BF16, tag=f"va{g}",
                              name=f"va_{g}") for g in range(tg_n)]
            at_g = [acts.tile([P, dc_n, QGW], BF16, tag=f"at{g}",
                              name=f"at_{g}") for g in range(tg_n)]

            _xdma_done = set()

            def load_x(name, g, x_sb, x_dram):
                """Load activations for one token group. Group 0 is split
                into half-K DMAs so the first projection chain starts
                sooner; later groups use one DMA to cut SP issue cost."""
                if (name, g) in _xdma_done:
                    return
                _xdma_done.add((name, g))
                src = x_dram[g]
                if g == 0:
                    for half in range(2):
                        k0 = half * khalf
                        nc.sync.dma_start(x_sb[:, k0:k0 + khalf, :],
                                          src[:, k0:k0 + khalf, :])
                else:
                    nc.sync.dma_start(x_sb[:], src)

            def proj_qk_units(tg, w_sb, x_sb, b_sb, dest, out, dcs=None):
                """Append filler units: 2 half-chains per dc."""
                cell = {}
                for dc in (range(dc_n) if dcs is None else dcs):
                    for half in range(2):
                        def chain(tg=tg, dc=dc, half=half, w_sb=w_sb,
                                  x_sb=x_sb, b_sb=b_sb, dest=dest):
                            if half == 0:
                                cell[dc] = ps.tile([P, QGW], F32, tag="pacc",
                                                   bufs=2,
                                                   name=f"pp_{tg}_{dc}")
                            pp = cell[dc]
                            k0 = half * khalf
                            for kc in range(k0, k0 + khalf):
                                nc.tensor.matmul(
                                    pp[:],
                                    lhsT=w_sb[:, kc, dc * P:(dc + 1) * P],
                                    rhs=x_sb[:, kc, :],
                                    start=(kc == 0),
                                    stop=(kc == kc_n - 1),
                                )
                            if half == 1:
                                if use_bias:
                                    nc.scalar.activation(
                                        dest[:, dc, :], pp[:], AF.Identity,
                                        bias=b_sb[:, dc:dc + 1])
                                else:
                                    nc.vector.tensor_copy(dest[:, dc, :],
                                                          pp[:])
                        out.append(chain)

            _xvt = {}

            def load_xv(tg):
                if tg in _xvt:
                    return
                xvt = xin.tile([P, kc_n, QGW], BF16, tag="xvstream",
                               bufs=2, name=f"xvt_{tg}")
                nc.sync.dma_start(xvt[:], xvT[tg])
                _xvt[tg] = xvt

            def proj_v_units(tg, out):
                def ones_unit(tg=tg):
                    load_xv(tg)
                    nc.gpsimd.memset(
                        va_g[tg].rearrange("p t (h e) -> p t h e",
                                           e=DH + 1)[:, :, :, DH], 1.0)
                out.append(ones_unit)
                for ti in range(tpg):
                    def v_unit(tg=tg, ti=ti):
                        xvt = _xvt[tg]
                        vp = ps.tile([P, QGW], F32, tag="pacc", bufs=2,
                                     name=f"vp_{tg}_{ti}")
                        if use_bias:
                            nc.tensor.matmul(vp[:, :hd],
                                             lhsT=onesb_sb[0:1, :],
                                             rhs=bv_sb[:, :], start=True,
                                             stop=False)
                        for kc in range(kc_n):
                            nc.tensor.matmul(
                                vp[:, :hd],
                                lhsT=xvt[:, kc, ti * P:(ti + 1) * P],
                                rhs=wv_sb[:, kc, :],
                                start=(kc == 0 and not use_bias),
                                stop=(kc == kc_n - 1),
                            )
                        nc.vector.tensor_copy(
                            va_g[tg][:, ti].rearrange(
                                "p (h e) -> p h e", e=DH + 1)[:, :, :DH],
                            vp[:, :hd].rearrange("p (h e) -> p h e", e=DH))
                    out.append(v_unit)

            def ff_units(qg, out, tail=False):
                for nck in range(kc_n):
                    def ff_unit(qg=qg, nck=nck, tail=tail):
                        fp = ps.tile([P, QGW], F32, tag="pacc", bufs=2,
                                     name=f"fp_{nck}_{qg}")
                        for dc in range(dc_n):
                            nc.tensor.matmul(
                                fp[:],
                                lhsT=wff_sb[:, dc, nck * P:(nck + 1) * P],
                                rhs=at_g[qg][:, dc, :],
                                start=(dc == 0),
                                stop=(dc == dc_n - 1),
                            )
                        ot = opool.tile([P, QGW], BF16, tag="otile",
                                        name=f"ot_{nck}_{qg}")
                        if use_bias:
                            nc.scalar.activation(ot[:], fp[:], AF.Identity,
                                                 bias=bffq_sb[:, nck:nck + 1])
                        elif tail:
                            # exp is done by now; use the idle scalar engine
                            nc.scalar.copy(ot[:], fp[:])
                        else:
                            nc.vector.tensor_copy(ot[:], fp[:])
                        nc.sync.dma_start(
                            outT[nck * P:(nck + 1) * P,
                                 qg * QGW:(qg + 1) * QGW], ot[:])
                    out.append(ff_unit)

            # two filler tiers: pf must drain before the next attention
            # group (projections); lf (ff) can slip to late groups.
            from collections import deque
            pf = deque()
            lf = deque()

            def run_units(n=None):
                k = (len(pf) + len(lf)) if n is None else n
                for _ in range(k):
                    if pf:
                        pf.popleft()()
                    elif lf:
                        lf.popleft()()
                    else:
                        return

            def drain_pf():
                while pf:
                    pf.popleft()()

            def attention(qg):
                kmax = (qg + 1) * tpg if variant == "causal" else tt
                for hp in range(hpc // 2):
                    dch = hp
                    h0 = 2 * hp
                    op_h = [
                        ps.tile([P, QGW], F32, tag="opacc", bufs=2,
                                name=f"op{j}_{hp}_{qg}")
                        for j in range(2)
                    ]
                    ets = {}

                    def emit_scores(kt, hp=hp, dch=dch, ets=ets, qg=qg):
                        off = (max(0, kt * P - qg * QGW)
                               if variant == "causal" else 0)
                        kg, kx = divmod(kt, tpg)
                        sp = ps.tile([P, 2, QGW], F32, tag="mmw", bufs=2,
                                     name=f"sp_{hp}_{qg}_{kt}")
                        for j in range(2):
                            r = j * DH
                            nc.tensor.matmul(
                                sp[:, j, off:],
                                lhsT=kT_g[kg][r:r + DH, dch,
                                              kx * P:(kx + 1) * P],
                                rhs=qT_g[qg][r:r + DH, dch, off:],
                                start=True,
                                stop=True,
                            )
                        if variant == "generic":
                            mb_sb = xin.tile([P, QGW], F32, tag="mstream",
                                             bufs=4,
                                             name=f"mb_{hp}_{qg}_{kt}")
                            nc.sync.dma_start(
                                mb_sb[:],
                                mbT[kt * P:(kt + 1) * P,
                                    qg * QGW:(qg + 1) * QGW])
                            for j in range(2):
                                nc.vector.tensor_add(
                                    sp[:, j, :], sp[:, j, :], mb_sb[:])
                        et = epool.tile([P, 2, QGW], BF16, tag="etile",
                                        name=f"et_{hp}_{qg}_{kt}")
                        # one ACTIVATE covers both heads even when the
                        # leading `off` columns are trimmed (3D AP)
                        nc.scalar.activation(et[:, :, off:], sp[:, :, off:],
                                             AF.Exp)
                        if variant == "causal" and kt * P - qg * QGW >= 0:
                            doff = kt * P - qg * QGW
                            for j in range(2):
                                nc.vector.tensor_mul(
                                    et[:, j, doff:doff + P],
                                    et[:, j, doff:doff + P],
                                    dmask_sb[:])
                        ets[kt] = (et, off)

                    def emit_av(kt, hp=hp, ets=ets, qg=qg, kmax=kmax,
                                op_h=op_h, h0=h0):
                        et, off = ets.pop(kt)
                        kg, kx = divmod(kt, tpg)
                        for j in range(2):
                            h = h0 + j
                            nc.tensor.matmul(
                                op_h[j][:DH + 1, off:],
                                lhsT=va_g[kg][:, kx, h * (DH + 1):
                                              (h + 1) * (DH + 1)],
                                rhs=et[:, j, off:],
                                start=(kt == 0),
                                stop=(kt == kmax - 1),
                            )

                    emit_scores(0)
                    for kt in range(1, kmax):
                        emit_scores(kt)
                        run_units(1)
                        emit_av(kt - 1)
                    emit_av(kmax - 1)
                    run_units(1)

                    # normalize: broadcast Z across partitions with a rank-1
                    # fp16 matmul, take 1/Z at full 128-lane width, and fuse
                    # the psum->sbuf cast with the multiply.
                    for j in range(2):
                        h = h0 + j
                        op = op_h[j]
                        po = (h * DH) % P
                        zrh = zp.tile([1, QGW], FP16, tag="zrh",
                                      name=f"zrh_{h}_{qg}")
                        nc.vector.tensor_copy(zrh[0:1, :], op[DH:DH + 1, :])
                        zbz = ps.tile([P, QGW], F32, tag="pacc", bufs=2,
                                      name=f"zbz_{h}_{qg}")
                        nc.tensor.matmul(
                            zbz[:],
                            lhsT=ones_sb[0:1, :],
                            rhs=zrh[0:1, :],
                            start=True, stop=True)
                        # custom-DVE ops ignore input base partitions, but
                        # this one reads a full base-0 tile (verified OK)
                        zbi = zp.tile([P, QGW], F32, tag="zbi", bufs=2,
                                      name=f"zbi_{h}_{qg}")
                        nc.vector.reciprocal_approx_fast(zbi[:], zbz[:])
                        nc.vector.tensor_mul(
                            at_g[qg][po:po + DH, dch, :],
                            op[:DH, :],
                            zbi[po:po + DH, :])

            # ---- schedule over token groups ----
            def queue_proj(tg, first=False):
                bqs = bq_sb if use_bias else None
                bks = bk_sb if use_bias else None
                if first:
                    # DMA transfers drain the queue serially, so issue in
                    # PE consumption order (q, k, then v) and use the dc-1
                    # chains to cover the v-transfer latency.
                    pf.append(lambda: load_w("wq", wq_sb, wqT, 0))
                    pf.append(lambda: load_w("wq", wq_sb, wqT, 1))
                    pf.append(lambda tg=tg: load_x("xq", tg, xq_g[tg], xqT))
                    pf.append(lambda: load_w("wk", wk_sb, wkT))
                    pf.append(lambda tg=tg: load_x("xk", tg, xk_g[tg], xkT))
                    pf.append(lambda: load_w("wv", wv_sb, wvT))
                    pf.append(lambda tg=tg: load_xv(tg))
                    proj_qk_units(tg, wq_sb, xq_g[tg], bqs, qT_g[tg], pf,
                                  dcs=(0,))
                    proj_qk_units(tg, wq_sb, xq_g[tg], bqs, qT_g[tg], pf,
                                  dcs=(1,))
                    proj_qk_units(tg, wk_sb, xk_g[tg], bks, kT_g[tg], pf,
                                  dcs=(0,))
                    proj_qk_units(tg, wk_sb, xk_g[tg], bks, kT_g[tg], pf,
                                  dcs=(1,))
                    proj_v_units(tg, pf)
                    return
                pf.append(lambda tg=tg: load_x("xq", tg, xq_g[tg], xqT))
                proj_qk_units(tg, wq_sb, xq_g[tg], bqs, qT_g[tg], pf)
                pf.append(lambda tg=tg: load_x("xk", tg, xk_g[tg], xkT))
                proj_qk_units(tg, wk_sb, xk_g[tg], bks, kT_g[tg], pf)
                proj_v_units(tg, pf)

            if variant == "causal":
                queue_proj(0, first=True)
                drain_pf()
                for tg in range(tg_n):
                    if tg + 1 < tg_n:
                        queue_proj(tg + 1)
                    lf.append(lambda: load_w("wff", wff_sb, wffT))
                    if tg > 0:
                        ff_units(tg - 1, lf)
                    attention(tg)
                    drain_pf()
                ff_units(tg_n - 1, lf, tail=True)
                run_units()
            else:
                queue_proj(0, first=True)
                for tg in range(1, tg_n):
                    queue_proj(tg)
                drain_pf()
                lf.append(lambda: load_w("wff", wff_sb, wffT))
                for qg in range(tg_n):
                    if qg > 0:
                        ff_units(qg - 1, lf)
                    attention(qg)
                ff_units(tg_n - 1, lf, tail=True)
                run_units()

    nc.compile()
    return nc


def _classify_mask(mask: np.ndarray) -> str:
    m = np.asarray(mask)[:, 0]  # [B, S, S]
    if not m.any():
        return "dense"
    s = m.shape[-1]
    causal = np.triu(np.ones((s, s), dtype=m.dtype), k=1)
    if all(np.array_equal(m[b], causal) for b in range(m.shape[0])):
        return "causal"
    return "generic"


def _bf(x):
    return np.ascontiguousarray(np.ascontiguousarray(x).astype(NPBF16))


def _tile_x(xT):
    """[d, s] -> [tg, P, kc, QGW] partition-major (8KB contiguous lines)."""
    kc = xT.shape[0] // P
    tg = xT.shape[1] // QGW
    return np.ascontiguousarray(
        xT.reshape(kc, P, tg, QGW).transpose(2, 1, 0, 3))


def _tile_w(wT):
    """[cP, m] -> [P, c, m] partition-major."""
    c = wT.shape[0] // P
    return np.ascontiguousarray(
        wT.reshape(c, P, wT.shape[1]).transpose(1, 0, 2))


def _make_in_maps(variant, query, key, value, mask, wq, bq, wk, bk, wv, bv,
                  wff, bff, use_bias):
    scale = np.float32(1.0 / np.sqrt(np.float32(DH)))
    wqTs = _bf((wq * scale).T)
    wkT = _bf(wk.T)
    wvT = _bf(wv.T)
    wffT = _bf(wff.T)

    qT = [_tile_x(_bf(query[b].T)) for b in range(B)]
    kT = [_tile_x(_bf(key[b].T)) for b in range(B)]
    vT = [_tile_x(_bf(value[b].T)) for b in range(B)]
    mbT = None
    if variant == "generic":
        mbT = [np.ascontiguousarray(mask[b, 0].T * np.float32(-1e9))
               for b in range(B)]

    dmask = np.tril(np.ones((P, P), np.float32)).T

    in_maps = []
    for c in range(NCORES):
        b, hg = c // GPB, c % GPB
        sl = slice(hg * HD, (hg + 1) * HD)
        m = {
            "xqT": qT[b], "xkT": kT[b], "xvT": vT[b],
            "wqT": _tile_w(np.ascontiguousarray(wqTs[:, sl])),
            "wkT": _tile_w(np.ascontiguousarray(wkT[:, sl])),
            "wvT": _tile_w(np.ascontiguousarray(wvT[:, sl])),
            "wffT": _tile_w(np.ascontiguousarray(wffT[sl, :])),
        }
        if use_bias:
            m["bq"] = np.ascontiguousarray((bq * scale)[sl]).astype(np.float32)
            m["bk"] = np.ascontiguousarray(bk[sl]).astype(np.float32)
            m["bv"] = _bf(bv[sl])[None, :]
            m["bffq"] = (bff / GPB).astype(np.float32)
            m["onesb"] = np.ones((1, P), NPBF16)
        if variant == "causal":
            m["dmask"] = _bf(dmask)
        if variant == "generic":
            m["mbT"] = mbT[b]
        in_maps.append(m)
    return in_maps


def kernel(**inputs) -> np.ndarray:
    query = np.ascontiguousarray(inputs["query"], dtype=np.float32)
    key = np.ascontiguousarray(inputs["key"], dtype=np.float32)
    value = np.ascontiguousarray(inputs["value"], dtype=np.float32)
    mask = np.asarray(inputs["mask"], dtype=np.float32)
    wq = np.asarray(inputs["wq"], np.float32)
    bq = np.asarray(inputs["bq"], np.float32)
    wk = np.asarray(inputs["wk"], np.float32)
    bk = np.asarray(inputs["bk"], np.float32)
    wv = np.asarray(inputs["wv"], np.float32)
    bv = np.asarray(inputs["bv"], np.float32)
    wff = np.asarray(inputs["wff"], np.float32)
    bff = np.asarray(inputs["bff"], np.float32)

    variant = _classify_mask(mask)
    use_bias = bool(bq.any() or bk.any() or bv.any() or bff.any())
    pkey = (variant, use_bias)
    if pkey not in _PROG_CACHE:
        _PROG_CACHE[pkey] = build_program(variant, use_bias)
    nc = _PROG_CACHE[pkey]

    in_maps = _make_in_maps(variant, query, key, value, mask, wq, bq, wk, bk,
                            wv, bv, wff, bff, use_bias)
    res = run_bass_kernel_spmd(nc, in_maps, core_ids=list(range(NCORES)))
    out = np.empty((B, S, D), np.float32)
    for b in range(B):
        acc = res.results[b * GPB]["outT"].astype(np.float32)
        for g in range(1, GPB):
            acc = acc + res.results[b * GPB + g]["outT"].astype(np.float32)
        out[b] = acc.T
    return out


if __name__ == "__main__":
    import reference

    inputs = {k: np.asarray(v) for k, v in reference.setup_inputs().items()}
    out = kernel(**inputs)
    print("kernel out:", out.shape, out.dtype)
